# revision 32
# baseline (speedup 1.0000x reference)
"""EntNet forward kernel for 8 Trainium2 NeuronCores (Bass/Tile).

Math note: in the reference, the gated memory is
    mem = memory_nodes * (1 + sigmoid(...))
followed by per-column L2 normalization.  Since (1 + sigmoid(x)) > 0 is a
per-column positive scalar, it cancels exactly in the normalization, so the
gate g — and with it s_in, F_i, input, keys, U, V, W, a_mem (cand is dead in
the source already) — does not affect the output.  Live computation:

    s_q = F_q @ query[0]                         # [D]
    mn  = memory_nodes / max(||col||_2, 1e-12)   # [D, M] column-normalized
    p   = softmax(s_q^T @ mn)                    # [1, M]
    u   = mn @ p^T                               # [D]
    y   = R @ prelu(s_q + H @ u, a_out)          # [D, 1]

Sharding: D is row-sharded over 8 cores (rows_c = 512c:512c+512).  Each core
streams its shards of F_q^T, mem, mem^T (row shards), H^T (H column shard),
R^T (R row shard) — ~18 MiB/core in bf16.  Collectives: a tiny warm-up
AllReduce at t~0 (absorbs CC cold start + inter-core launch skew), a 4 KB
AllReduce of the r = mem^T s_q partials, and a 16 KB AllReduce of
z = s_q + H@u partials.

Precision: all matrices ship as plain bf16 (half the f32 bytes).  The final
relative error is ~3e-3 (measured vs the f32 reference), dominated by bf16
rounding of F_q/H/R; the softmax is effectively one-hot (top-2 logit gap
~80), so the p path is insensitive.  PSUM accumulation is f32.

Vectors are partition-major throughout: v[128i + p] <-> tile[p, i].
"""

import sys
import numpy as np

for _p in ("/root/.axon_site/_ro/trn_rl_repo", "/opt/trn_rl_repo"):
    if _p not in sys.path:
        sys.path.append(_p)

D, M, L = 4096, 1024, 8192
N_CORES = 8

_CACHE = {}


def _build_module(n_cores, d, m, l):  # noqa: E741
    import concourse.bacc as bacc
    import concourse.tile as tile
    import concourse.tile as tile_mod
    import concourse.mybir as mybir
    import concourse.bass_isa as bass_isa

    f32 = mybir.dt.float32
    bf16 = mybir.dt.bfloat16
    DL = d // n_cores      # local rows of D
    KD = DL // 128         # local d chunks
    KM = m // 128          # m chunks
    KZ = d // 128          # global d chunks
    KL = l // 128          # l chunks
    AF = mybir.ActivationFunctionType
    ADD = mybir.AluOpType.add
    rg = [list(range(n_cores))]

    # F_q^T streamed in chunks of FQ_N l-tiles (8 -> 1 MiB bf16 at full size)
    FQ_N = min(8, KL)
    assert KL % FQ_N == 0
    FQCH = KL // FQ_N

    nc = bacc.Bacc("TRN2", target_bir_lowering=False, debug=False,
                   enable_asserts=False, num_devices=n_cores)

    fqT_in = nc.dram_tensor("fqT", [FQCH * 128, FQ_N * DL], bf16,
                            kind="ExternalInput")
    q2d_in = nc.dram_tensor("q2d", [128, KL], bf16, kind="ExternalInput")
    memd_in = nc.dram_tensor("memd", [128, KD * m], bf16, kind="ExternalInput")
    memT_in = nc.dram_tensor("memT", [128, KM * DL], bf16, kind="ExternalInput")
    hT_in = nc.dram_tensor("hT", [128, KD * d], bf16, kind="ExternalInput")
    rT_in = nc.dram_tensor("rT", [128, KZ * DL], bf16, kind="ExternalInput")
    ab_in = nc.dram_tensor("ab", [128, 1], f32, kind="ExternalInput")
    mask_in = nc.dram_tensor("mask", [128, KZ], f32, kind="ExternalInput")
    y_out = nc.dram_tensor("y", [128, KD], f32, kind="ExternalOutput")
    hw_out = nc.dram_tensor("hw", [1, 8], f32, kind="ExternalOutput")

    with tile.TileContext(nc) as tc:
        with (
            tc.tile_pool(name="consts", bufs=1) as consts,
            tc.tile_pool(name="fq", bufs=6) as fqp,
            tc.tile_pool(name="mem", bufs=1) as memp,
            tc.tile_pool(name="big", bufs=1) as bigp,
            tc.tile_pool(name="sm", bufs=1) as smp,
            tc.tile_pool(name="scr", bufs=2) as scrp,
            tc.tile_pool(name="ps_sq", bufs=1, space="PSUM") as ps_sq,
            tc.tile_pool(name="ps_r", bufs=1, space="PSUM") as ps_r,
            tc.tile_pool(name="ps_u", bufs=1, space="PSUM") as ps_u,
            tc.tile_pool(name="ps_z", bufs=1, space="PSUM") as ps_z,
            tc.tile_pool(name="ps_y", bufs=1, space="PSUM") as ps_y,
            tc.tile_pool(name="ps_h", bufs=1, space="PSUM") as ps_h,
            tc.tile_pool(name="dram", bufs=1, space="DRAM") as dram,
        ):
            # ---- small constants (gpsimd queue, land in ~1us) ----
            q2d = consts.tile([128, KL], bf16)
            nc.gpsimd.dma_start(q2d[:], q2d_in[:])
            ab = consts.tile([128, 1], f32)
            nc.gpsimd.dma_start(ab[:], ab_in[:])
            mask = consts.tile([128, KZ], f32)
            nc.gpsimd.dma_start(mask[:], mask_in[:])

            BYP = mybir.AluOpType.bypass
            pw = consts.tile([128, 1], f32)
            nc.gpsimd.memset(pw[:], 1.0)

            # ---- phase 1: s_q = F_q @ query, streaming F_q^T over 3 queues ----
            psq = ps_sq.tile([128, KD], f32)
            fqT_r = fqT_in[:].rearrange("(i p) x -> p i x", p=128)
            engs = [nc.sync, nc.scalar]
            n_mm = 0
            N_MM_TOT = KL * KD
            for i in range(FQCH):
                fq_t = fqp.tile([128, FQ_N, DL], bf16)
                engs[i % len(engs)].dma_start(
                    fq_t[:].rearrange("p a b -> p (a b)"), fqT_r[:, i, :])
                for j in range(FQ_N):
                    n = FQ_N * i + j
                    for dt in range(KD):
                        nc.tensor.matmul(
                            psq[:, dt:dt + 1],
                            fq_t[:, j, 128 * dt:128 * (dt + 1)],
                            q2d[:, n:n + 1],
                            start=(n_mm == 0), stop=(n_mm == N_MM_TOT - 1),
                        )
                        n_mm += 1
            # mem shards go behind the last gpsimd fq trigger; H^T and R^T
            # land after the F stream on the two big queues.
            memd = memp.tile([128, KD, m], bf16)
            memT = memp.tile([128, KM, DL], bf16)
            md_op = nc.gpsimd.dma_start(
                memd[:].rearrange("p a b -> p (a b)"), memd_in[:])
            mt_op = nc.gpsimd.dma_start(
                memT[:].rearrange("p a b -> p (a b)"), memT_in[:])
            # warm the GpSimd partition-reduce ucode (first dispatch stalls
            # the engine ~20us) — after the memd/memT triggers so the stall
            # cannot delay those descriptors
            pw2 = consts.tile([128, 1], f32)
            pw2_op = nc.gpsimd.partition_all_reduce(pw2[:], pw[:], 128,
                                                    bass_isa.ReduceOp.max)
            tile_mod.add_dep_helper(pw2_op.ins, mt_op.ins, sync=False,
                                    reason="gpsimd warmups after mem DMAs")
            pw3 = consts.tile([128, 1], f32)
            nc.gpsimd.partition_all_reduce(pw3[:], pw2[:], 128,
                                           bass_isa.ReduceOp.add)
            hT = bigp.tile([128, KD, d], bf16)
            nc.sync.dma_start(hT[:].rearrange("p a b -> p (a b)"), hT_in[:])
            rT = bigp.tile([128, KZ, DL], bf16)
            nc.scalar.dma_start(rT[:].rearrange("p a b -> p (a b)"), rT_in[:])

            # ---- ACT table warmup (after the scalar-queue fq triggers; the
            # tables just have to be resident before the Exp at softmax time)
            warm = consts.tile([1, 1], f32)
            nc.gpsimd.memset(warm[:], 1.0)
            w2 = consts.tile([1, 1], f32)
            nc.scalar.activation(w2[:], warm[:], AF.Square)
            nc.scalar.activation(w2[:], warm[:], AF.Sqrt)
            nc.scalar.activation(w2[:], warm[:], AF.Exp)
            nc.scalar.activation(w2[:], warm[:], AF.Relu)

            s_q = smp.tile([128, KD], f32)
            nc.vector.tensor_copy(s_q[:], psq[:])
            sq_b = smp.tile([128, KD], bf16)
            nc.vector.tensor_copy(sq_b[:], s_q[:])
            # s_q placed into its full-D slot (via mask) while AR1 runs
            sqm = smp.tile([128, KZ], f32)
            nc.vector.tensor_tensor(
                sqm[:].rearrange("p (r k) -> p r k", k=KD),
                s_q[:].unsqueeze(1).broadcast_to([128, KZ // KD, KD]),
                mask[:].rearrange("p (r k) -> p r k", k=KD),
                mybir.AluOpType.mult,
            )

            # ---- r = mem^T @ s_q (local-d partial) ----
            pr = ps_r.tile([128, KM], f32)
            n_mm = 0
            N_MM_TOT = KM * KD
            for mt in range(KM):
                for kc in range(KD):
                    last_r_mm = nc.tensor.matmul(
                        pr[:, mt:mt + 1],
                        memd[:, kc, 128 * mt:128 * (mt + 1)],
                        sq_b[:, kc:kc + 1],
                        start=(n_mm == 0), stop=(n_mm == N_MM_TOT - 1),
                    )
                    n_mm += 1

            # PE heater: keep the PE warm through the r-AllReduce wait
            HW_N = min(512, DL)
            ph = ps_h.tile([1, HW_N], f32)
            h1 = []
            for k in range(30):
                h1.append(nc.tensor.matmul(
                    ph[:, :], memT[:, 0, k:k + 1], memT[:, 0, 0:HW_N],
                    start=(k == 0), stop=(k == 29)))
            tile_mod.add_dep_helper(h1[0].ins, last_r_mm.ins, sync=False,
                                    reason="heater after r")

            # ---- AllGather r partials, then local tree-sum (an AllGather is
            # one mesh phase instead of AllReduce's two) ----
            r_sb = smp.tile([128, KM], f32)
            nc.vector.tensor_copy(r_sb[:], pr[:])
            cr_i = dram.tile([128, KM], f32)
            cr_o = dram.tile([n_cores * 128, KM], f32)
            nc.gpsimd.dma_start(cr_i[:], r_sb[:])
            nc.gpsimd.collective_compute(
                "AllGather", BYP, replica_groups=rg,
                ins=[cr_i[:].opt()], outs=[cr_o[:].opt()])
            rg8 = smp.tile([128, n_cores, KM], f32)
            nc.gpsimd.dma_start(
                rg8[:], cr_o[:].rearrange("(g p) k -> p g k", p=128))
            rg8f = rg8[:].rearrange("p g k -> p (g k)")
            r4 = smp.tile([128, 4 * KM], f32)
            nc.vector.tensor_add(r4[:], rg8f[:, 0:4 * KM], rg8f[:, 4 * KM:])
            r2 = smp.tile([128, 2 * KM], f32)
            nc.vector.tensor_add(r2[:], r4[:, 0:2 * KM], r4[:, 2 * KM:])
            rf = smp.tile([128, KM], f32)
            nc.vector.tensor_add(rf[:], r2[:, 0:KM], r2[:, KM:])

            # ---- softmax (partition-major); the column norms are folded
            # into memd/memT on the host, so rf already holds the logits ----
            tm = smp.tile([128, 1], f32)
            nc.vector.tensor_reduce(tm[:], rf[:], mybir.AxisListType.X,
                                    mybir.AluOpType.max)
            tmb = smp.tile([128, 1], f32)
            nc.gpsimd.partition_all_reduce(tmb[:], tm[:], 128,
                                           bass_isa.ReduceOp.max)
            negmx = smp.tile([128, 1], f32)
            nc.vector.tensor_scalar_mul(negmx[:], tmb[:], -1.0)
            e = smp.tile([128, KM], f32)
            esum = smp.tile([128, 1], f32)
            nc.scalar.activation(e[:], rf[:], AF.Exp, bias=negmx[:],
                                 accum_out=esum[:])
            esb = smp.tile([128, 1], f32)
            nc.gpsimd.partition_all_reduce(esb[:], esum[:], 128,
                                           bass_isa.ReduceOp.add)
            rsb = smp.tile([128, 1], f32)
            nc.vector.reciprocal(rsb[:], esb[:])
            pt = smp.tile([128, KM], f32)
            nc.vector.tensor_scalar_mul(pt[:], e[:], rsb[:])
            pt_b = smp.tile([128, KM], bf16)
            nc.vector.tensor_copy(pt_b[:], pt[:])

            # ---- u = mem @ (p/denom), local rows ----
            pu = ps_u.tile([128, KD], f32)
            n_mm = 0
            N_MM_TOT = KD * KM
            for dt in range(KD):
                for kc in range(KM):
                    nc.tensor.matmul(
                        pu[:, dt:dt + 1],
                        memT[:, kc, 128 * dt:128 * (dt + 1)],
                        pt_b[:, kc:kc + 1],
                        start=(n_mm == 0), stop=(n_mm == N_MM_TOT - 1),
                    )
                    n_mm += 1
            u_b = smp.tile([128, KD], bf16)
            nc.vector.tensor_copy(u_b[:], pu[:])

            # ---- z partial = H[:, cols_c] @ u_c (full-D, partition-major) ----
            pz = ps_z.tile([128, KZ], f32)
            n_mm = 0
            N_MM_TOT = KD * KZ
            for dt in range(KZ):
                for kc in range(KD):
                    last_z_mm = nc.tensor.matmul(
                        pz[:, dt:dt + 1],
                        hT[:, kc, 128 * dt:128 * (dt + 1)],
                        u_b[:, kc:kc + 1],
                        start=(n_mm == 0), stop=(n_mm == N_MM_TOT - 1),
                    )
                    n_mm += 1
            h2 = []
            for k in range(52):
                h2.append(nc.tensor.matmul(
                    ph[:, :], memT[:, 0, k:k + 1], memT[:, 0, 0:HW_N],
                    start=(k == 0), stop=(k == 51)))
            tile_mod.add_dep_helper(h2[0].ins, last_z_mm.ins, sync=False,
                                    reason="heater after z")

            # add this core's s_q (pre-placed in sqm during the AR1 wait)
            ar2 = smp.tile([128, KZ], f32)
            nc.vector.tensor_add(ar2[:], sqm[:], pz[:])

            # ---- AllGather z partials, local tree-sum ----
            car2_i = dram.tile([128, KZ], f32)
            car2_o = dram.tile([n_cores * 128, KZ], f32)
            nc.gpsimd.dma_start(car2_i[:], ar2[:])
            nc.gpsimd.collective_compute(
                "AllGather", BYP, replica_groups=rg,
                ins=[car2_i[:].opt()], outs=[car2_o[:].opt()])
            zg8 = smp.tile([128, n_cores, KZ], f32)
            nc.gpsimd.dma_start(
                zg8[:], car2_o[:].rearrange("(g p) k -> p g k", p=128))
            zg8f = zg8[:].rearrange("p g k -> p (g k)")
            z4 = smp.tile([128, 4 * KZ], f32)
            nc.vector.tensor_add(z4[:], zg8f[:, 0:4 * KZ], zg8f[:, 4 * KZ:])
            z2 = smp.tile([128, 2 * KZ], f32)
            nc.vector.tensor_add(z2[:], z4[:, 0:2 * KZ], z4[:, 2 * KZ:])
            zf = smp.tile([128, KZ], f32)
            nc.vector.tensor_add(zf[:], z2[:, 0:KZ], z2[:, KZ:])

            # ---- prelu(z) = max(z,0) + a*min(z,0), all on the DVE ----
            pos = smp.tile([128, KZ], f32)
            nc.vector.tensor_scalar_max(pos[:], zf[:], 0.0)
            negs = smp.tile([128, KZ], f32)
            nc.vector.tensor_scalar(negs[:], zf[:], 0.0, ab[:],
                                    mybir.AluOpType.min,
                                    mybir.AluOpType.mult)
            pzz = smp.tile([128, KZ], f32)
            nc.vector.tensor_add(pzz[:], pos[:], negs[:])
            pz_b = smp.tile([128, KZ], bf16)
            pzb_op = nc.vector.tensor_copy(pz_b[:], pzz[:])

            # ---- y = R[rows_c] @ prelu(z): partition-major out ----
            py = ps_y.tile([128, KD], f32)
            n_mm = 0
            N_MM_TOT = KZ * KD
            for kc in range(KZ):
                for dt in range(KD):
                    nc.tensor.matmul(
                        py[:, dt:dt + 1],
                        rT[:, kc, 128 * dt:128 * (dt + 1)],
                        pz_b[:, kc:kc + 1],
                        start=(n_mm == 0), stop=(n_mm == N_MM_TOT - 1),
                    )
                    n_mm += 1

            # consume heater + warmup results (anti-DCE) via dummy output.
            # Pinned after the pz_b cast so they run on the DVE during the y
            # matmuls instead of serializing after the y copy.
            hw_sb = smp.tile([1, 8], f32)
            hw0 = nc.vector.memset(hw_sb[:], 0.0)
            hw1 = nc.vector.tensor_copy(hw_sb[:, 0:4], ph[0:1, 0:4])
            hw3 = nc.vector.tensor_copy(hw_sb[:, 5:6], pw3[0:1, 0:1])
            for hw_op in (hw0, hw1, hw3):
                tile_mod.add_dep_helper(hw_op.ins, pzb_op.ins, sync=False,
                                        reason="anti-DCE copies during y")
            nc.sync.dma_start(hw_out[:], hw_sb[:])

            y_sb = smp.tile([128, KD], f32)
            nc.vector.tensor_copy(y_sb[:], py[:])
            nc.sync.dma_start(y_out[:], y_sb[:])

    nc.compile()
    return nc


def _get_module(n_cores=N_CORES, d=D, m=M, l=L):  # noqa: E741
    key = (n_cores, d, m, l)
    if key not in _CACHE:
        _CACHE[key] = _build_module(n_cores, d, m, l)
    return _CACHE[key]


def _bf(x):
    import ml_dtypes
    return np.ascontiguousarray(x).astype(ml_dtypes.bfloat16)


def _pack(x, group):
    """[n*128, e] -> [128, ...] per-partition-contiguous: rows grouped into
    chunks of `group` 128-row tiles laid side by side along the free dim."""
    n128, e = x.shape
    n = n128 // 128
    assert n % group == 0
    return np.ascontiguousarray(
        x.reshape(n // group, group, 128, e).transpose(0, 2, 1, 3)
    ).reshape((n // group) * 128, group * e)


def _make_in_maps(n_cores, d, m, l, F_q, query, memory_nodes, H, R, a_out):  # noqa: E741
    f32 = np.float32
    DL = d // n_cores
    KZ = d // 128
    KD = DL // 128
    KL = l // 128
    q2d = np.ascontiguousarray(query.reshape(KL, 128).T).astype(f32, copy=False)
    ss_full = (memory_nodes.astype(np.float64)**2).sum(axis=0).astype(f32)
    rdn = 1.0 / np.maximum(np.sqrt(ss_full), 1e-12)
    mem_hat = (memory_nodes * rdn[None, :]).astype(f32)
    FQ_N = min(8, KL)
    in_maps = []
    for c in range(n_cores):
        rows = slice(DL * c, DL * (c + 1))
        mask = np.zeros((128, KZ), f32)
        mask[:, KD * c:KD * (c + 1)] = 1.0
        in_maps.append({
            "fqT": _pack(_bf(F_q[rows].T), FQ_N),
            "q2d": _bf(q2d),
            "memd": _pack(_bf(mem_hat[rows]), DL // 128),
            "memT": _pack(_bf(mem_hat[rows].T), m // 128),
            "hT": _pack(_bf(H[:, rows].T), DL // 128),
            "rT": _pack(_bf(R[rows].T), d // 128),
            "ab": np.full((128, 1), a_out, f32),
            "mask": mask,
        })
    return in_maps


class _PjrtRunner:
    """Cached jit(shard_map(bass_exec)) so repeat kernel() calls skip
    retracing/recompiling (bass_utils.run_bass_kernel_spmd rebuilds the jit
    closure every call)."""

    def __init__(self, nc, n_cores):
        import jax
        from jax.sharding import Mesh, PartitionSpec
        from jax.experimental.shard_map import shard_map
        from concourse import bass2jax
        import concourse.mybir as mybir

        bass2jax.install_neuronx_cc_hook()
        self.n_cores = n_cores
        part_name = (nc.partition_id_tensor.name
                     if nc.partition_id_tensor else None)
        in_names, out_names, out_avals = [], [], []
        for alloc in nc.m.functions[0].allocations:
            if not isinstance(alloc, mybir.MemoryLocationSet):
                continue
            name = alloc.memorylocations[0].name
            if alloc.kind == "ExternalInput":
                if name != part_name:
                    in_names.append(name)
            elif alloc.kind == "ExternalOutput":
                out_names.append(name)
                out_avals.append(jax.core.ShapedArray(
                    tuple(alloc.tensor_shape), mybir.dt.np(alloc.dtype)))
        self.in_names, self.out_names, self.out_avals = in_names, out_names, out_avals
        n_params = len(in_names)
        self.zero_outs = [np.zeros(a.shape, a.dtype) for a in out_avals]
        all_in_names = tuple(in_names + out_names)
        if part_name is not None:
            all_in_names = all_in_names + (part_name,)

        def _body(*args):
            operands = list(args)
            if part_name is not None:
                operands.append(bass2jax.partition_id_tensor())
            outs = bass2jax._bass_exec_p.bind(
                *operands,
                out_avals=tuple(out_avals),
                in_names=all_in_names,
                out_names=tuple(out_names),
                lowering_input_output_aliases=(),
                sim_require_finite=True,
                sim_require_nnan=True,
                nc=nc,
            )
            return tuple(outs)

        devices = jax.devices()[:n_cores]
        mesh = Mesh(np.asarray(devices), ("core",))
        n_out = len(out_names)
        self._fn = jax.jit(
            shard_map(
                _body, mesh=mesh,
                in_specs=(PartitionSpec("core"),) * (n_params + n_out),
                out_specs=(PartitionSpec("core"),) * n_out,
                check_rep=False,
            ),
            keep_unused=True,
        )

    def __call__(self, in_maps):
        n = self.n_cores
        concat_in = [
            np.concatenate([in_maps[c][name] for c in range(n)], axis=0)
            for name in self.in_names
        ]
        concat_zeros = [
            np.zeros((n * z.shape[0], *z.shape[1:]), z.dtype)
            for z in self.zero_outs
        ]
        out_arrs = self._fn(*concat_in, *concat_zeros)
        return [
            {name: np.asarray(out_arrs[i]).reshape(n, *self.out_avals[i].shape)[c]
             for i, name in enumerate(self.out_names)}
            for c in range(n)
        ]


_RUNNER = {}


def _get_runner():
    if "r" not in _RUNNER:
        _RUNNER["r"] = _PjrtRunner(_get_module(), N_CORES)
    return _RUNNER["r"]


def kernel(**inputs):
    f32 = np.float32
    F_q = np.asarray(inputs["F_q"], f32)
    query = np.asarray(inputs["query"], f32).reshape(-1)
    memory_nodes = np.asarray(inputs["memory_nodes"], f32)
    H = np.asarray(inputs["H"], f32)
    R = np.asarray(inputs["R"], f32)
    a_out = float(np.asarray(inputs["a_out"]).reshape(-1)[0])

    in_maps = _make_in_maps(N_CORES, D, M, L, F_q, query, memory_nodes,
                            H, R, a_out)
    results = _get_runner()(in_maps)
    y = np.concatenate(
        [np.ascontiguousarray(results[c]["y"].T).reshape(-1)
         for c in range(N_CORES)])
    return y.reshape(D, 1).astype(f32)


# revision 42
# speedup vs baseline: 1.0490x; 1.0490x over previous
"""EntNet forward kernel for 8 Trainium2 NeuronCores (Bass/Tile).

Math note: in the reference, the gated memory is
    mem = memory_nodes * (1 + sigmoid(...))
followed by per-column L2 normalization.  Since (1 + sigmoid(x)) > 0 is a
per-column positive scalar, it cancels exactly in the normalization, so the
gate g — and with it s_in, F_i, input, keys, U, V, W, a_mem (cand is dead in
the source already) — does not affect the output.  Live computation:

    s_q = F_q @ query[0]                         # [D]
    mn  = memory_nodes / max(||col||_2, 1e-12)   # [D, M] column-normalized
    p   = softmax(s_q^T @ mn)                    # [1, M]
    u   = mn @ p^T                               # [D]
    y   = R @ prelu(s_q + H @ u, a_out)          # [D, 1]

Sharding: D is row-sharded over 8 cores (rows_c = 512c:512c+512).  Each core
streams its shards of F_q^T, mem, mem^T (row shards), H^T (H column shard),
R^T (R row shard) — ~18 MiB/core in bf16.  Collectives: a tiny warm-up
AllReduce at t~0 (absorbs CC cold start + inter-core launch skew), a 4 KB
AllReduce of the r = mem^T s_q partials, and a 16 KB AllReduce of
z = s_q + H@u partials.

Precision: all matrices ship as plain bf16 (half the f32 bytes).  The final
relative error is ~3e-3 (measured vs the f32 reference), dominated by bf16
rounding of F_q/H/R; the softmax is effectively one-hot (top-2 logit gap
~80), so the p path is insensitive.  PSUM accumulation is f32.

Vectors are partition-major throughout: v[128i + p] <-> tile[p, i].
"""

import sys
import numpy as np

for _p in ("/root/.axon_site/_ro/trn_rl_repo", "/opt/trn_rl_repo"):
    if _p not in sys.path:
        sys.path.append(_p)

D, M, L = 4096, 1024, 8192
N_CORES = 8

_CACHE = {}


def _build_module(n_cores, d, m, l):  # noqa: E741
    import concourse.bacc as bacc
    import concourse.tile as tile
    import concourse.tile as tile_mod
    import concourse.mybir as mybir
    import concourse.bass_isa as bass_isa

    f32 = mybir.dt.float32
    bf16 = mybir.dt.bfloat16
    DL = d // n_cores      # local rows of D
    KD = DL // 128         # local d chunks
    KM = m // 128          # m chunks
    KZ = d // 128          # global d chunks
    KL = l // 128          # l chunks
    AF = mybir.ActivationFunctionType
    ADD = mybir.AluOpType.add
    rg = [list(range(n_cores))]

    # F_q^T streamed in chunks of FQ_N l-tiles (8 -> 1 MiB bf16 at full size)
    FQ_N = min(8, KL)
    assert KL % FQ_N == 0
    FQCH = KL // FQ_N

    nc = bacc.Bacc("TRN2", target_bir_lowering=False, debug=False,
                   enable_asserts=False, num_devices=n_cores)

    fqT_in = nc.dram_tensor("fqT", [FQCH * 128, FQ_N * DL], bf16,
                            kind="ExternalInput")
    q2d_in = nc.dram_tensor("q2d", [128, KL], bf16, kind="ExternalInput")
    memd_in = nc.dram_tensor("memd", [128, KD * m], bf16, kind="ExternalInput")
    # mem_hat^T replicated in full: u is computed on every core, so the
    # z/prelu/y chain is local and the second collective disappears
    memT_in = nc.dram_tensor("memT", [128, KM * d], bf16, kind="ExternalInput")
    hT_in = nc.dram_tensor("hT", [128, KZ * DL], bf16, kind="ExternalInput")
    rT_in = nc.dram_tensor("rT", [128, KD * d], bf16, kind="ExternalInput")
    ab_in = nc.dram_tensor("ab", [128, 1], f32, kind="ExternalInput")
    y_out = nc.dram_tensor("y", [128, KZ], f32, kind="ExternalOutput")
    hw_out = nc.dram_tensor("hw", [1, 8], f32, kind="ExternalOutput")

    with tile.TileContext(nc) as tc:
        with (
            tc.tile_pool(name="consts", bufs=1) as consts,
            tc.tile_pool(name="fq", bufs=4) as fqp,
            tc.tile_pool(name="mem", bufs=1) as memp,
            tc.tile_pool(name="big", bufs=1) as bigp,
            tc.tile_pool(name="sm", bufs=1) as smp,
            tc.tile_pool(name="scr", bufs=2) as scrp,
            tc.tile_pool(name="ps_sq", bufs=1, space="PSUM") as ps_sq,
            tc.tile_pool(name="ps_r", bufs=1, space="PSUM") as ps_r,
            tc.tile_pool(name="ps_u", bufs=1, space="PSUM") as ps_u,
            tc.tile_pool(name="ps_z", bufs=1, space="PSUM") as ps_z,
            tc.tile_pool(name="ps_y", bufs=1, space="PSUM") as ps_y,
            tc.tile_pool(name="ps_h", bufs=1, space="PSUM") as ps_h,
            tc.tile_pool(name="dram", bufs=1, space="DRAM") as dram,
        ):
            # ---- small constants (gpsimd queue, land in ~1us) ----
            q2d = consts.tile([128, KL], bf16)
            nc.gpsimd.dma_start(q2d[:], q2d_in[:])
            ab = consts.tile([128, 1], f32)
            nc.gpsimd.dma_start(ab[:], ab_in[:])

            BYP = mybir.AluOpType.bypass
            pw = consts.tile([128, 1], f32)
            nc.gpsimd.memset(pw[:], 1.0)

            # ---- phase 1: s_q = F_q @ query, streaming F_q^T over 3 queues ----
            psq = ps_sq.tile([128, KD], f32)
            fqT_r = fqT_in[:].rearrange("(i p) x -> p i x", p=128)
            engs = [nc.sync, nc.scalar]
            n_mm = 0
            N_MM_TOT = KL * KD
            for i in range(FQCH):
                fq_t = fqp.tile([128, FQ_N, DL], bf16)
                engs[i % len(engs)].dma_start(
                    fq_t[:].rearrange("p a b -> p (a b)"), fqT_r[:, i, :])
                for j in range(FQ_N):
                    n = FQ_N * i + j
                    for dt in range(KD):
                        nc.tensor.matmul(
                            psq[:, dt:dt + 1],
                            fq_t[:, j, 128 * dt:128 * (dt + 1)],
                            q2d[:, n:n + 1],
                            start=(n_mm == 0), stop=(n_mm == N_MM_TOT - 1),
                        )
                        n_mm += 1
            # mem row-shard goes behind the last gpsimd fq trigger; H^T, R^T
            # and the replicated mem_hat^T land after the F stream on the two
            # big queues (memT split across both).
            memd = memp.tile([128, KD, m], bf16)
            memT = memp.tile([128, KM, d], bf16)
            md_op = nc.gpsimd.dma_start(
                memd[:].rearrange("p a b -> p (a b)"), memd_in[:])
            # warm the GpSimd partition-reduce ucode (first dispatch stalls
            # the engine ~20us) — after the memd trigger so the stall cannot
            # delay that descriptor
            pw2 = consts.tile([128, 1], f32)
            pw2_op = nc.gpsimd.partition_all_reduce(pw2[:], pw[:], 128,
                                                    bass_isa.ReduceOp.max)
            tile_mod.add_dep_helper(pw2_op.ins, md_op.ins, sync=False,
                                    reason="gpsimd warmups after mem DMA")
            pw3 = consts.tile([128, 1], f32)
            nc.gpsimd.partition_all_reduce(pw3[:], pw2[:], 128,
                                           bass_isa.ReduceOp.add)
            hT = bigp.tile([128, KZ, DL], bf16)
            nc.sync.dma_start(hT[:].rearrange("p a b -> p (a b)"), hT_in[:])
            rT = bigp.tile([128, KD, d], bf16)
            nc.scalar.dma_start(rT[:].rearrange("p a b -> p (a b)"), rT_in[:])
            memT_f = memT[:].rearrange("p a b -> p (a b)")
            HALF = KM * d // 2
            nc.sync.dma_start(memT_f[:, 0:HALF], memT_in[:, 0:HALF])
            nc.scalar.dma_start(memT_f[:, HALF:], memT_in[:, HALF:])

            # ---- ACT table warmup (after the scalar-queue fq triggers; the
            # tables just have to be resident before the Exp at softmax time)
            warm = consts.tile([1, 1], f32)
            nc.gpsimd.memset(warm[:], 1.0)
            w2 = consts.tile([1, 1], f32)
            nc.scalar.activation(w2[:], warm[:], AF.Square)
            nc.scalar.activation(w2[:], warm[:], AF.Sqrt)
            nc.scalar.activation(w2[:], warm[:], AF.Exp)
            nc.scalar.activation(w2[:], warm[:], AF.Relu)

            s_q = smp.tile([128, KD], f32)
            nc.vector.tensor_copy(s_q[:], psq[:])
            sq_b = smp.tile([128, KD], bf16)
            nc.vector.tensor_copy(sq_b[:], s_q[:])

            # ---- r = mem^T @ s_q (local-d partial) ----
            pr = ps_r.tile([128, KM], f32)
            n_mm = 0
            N_MM_TOT = KM * KD
            for mt in range(KM):
                for kc in range(KD):
                    last_r_mm = nc.tensor.matmul(
                        pr[:, mt:mt + 1],
                        memd[:, kc, 128 * mt:128 * (mt + 1)],
                        sq_b[:, kc:kc + 1],
                        start=(n_mm == 0), stop=(n_mm == N_MM_TOT - 1),
                    )
                    n_mm += 1

            # PE heater: keep the PE warm through the r-AllGather wait
            HW_N = min(512, m)
            ph = ps_h.tile([1, HW_N], f32)
            h1 = []
            for k in range(30):
                h1.append(nc.tensor.matmul(
                    ph[:, :], memd[:, 0, k:k + 1], memd[:, 0, 0:HW_N],
                    start=(k == 0), stop=(k == 29)))
            tile_mod.add_dep_helper(h1[0].ins, last_r_mm.ins, sync=False,
                                    reason="heater after r")

            # ---- AllGather r partials, then local tree-sum (an AllGather is
            # one mesh phase instead of AllReduce's two) ----
            r_sb = smp.tile([128, KM], f32)
            nc.vector.tensor_copy(r_sb[:], pr[:])
            cr_i = dram.tile([128, KM], f32)
            cr_o = dram.tile([n_cores * 128, KM], f32)
            nc.gpsimd.dma_start(cr_i[:], r_sb[:])
            nc.gpsimd.collective_compute(
                "AllGather", BYP, replica_groups=rg,
                ins=[cr_i[:].opt()], outs=[cr_o[:].opt()])
            rg8 = smp.tile([128, n_cores, KM], f32)
            nc.gpsimd.dma_start(
                rg8[:], cr_o[:].rearrange("(g p) k -> p g k", p=128))
            rg8f = rg8[:].rearrange("p g k -> p (g k)")
            r4 = smp.tile([128, 4 * KM], f32)
            nc.vector.tensor_add(r4[:], rg8f[:, 0:4 * KM], rg8f[:, 4 * KM:])
            r2 = smp.tile([128, 2 * KM], f32)
            nc.vector.tensor_add(r2[:], r4[:, 0:2 * KM], r4[:, 2 * KM:])
            rf = smp.tile([128, KM], f32)
            nc.vector.tensor_add(rf[:], r2[:, 0:KM], r2[:, KM:])

            # ---- softmax (partition-major); the column norms are folded
            # into memd/memT on the host, so rf already holds the logits ----
            tm = smp.tile([128, 1], f32)
            nc.vector.tensor_reduce(tm[:], rf[:], mybir.AxisListType.X,
                                    mybir.AluOpType.max)
            tmb = smp.tile([128, 1], f32)
            nc.gpsimd.partition_all_reduce(tmb[:], tm[:], 128,
                                           bass_isa.ReduceOp.max)
            negmx = smp.tile([128, 1], f32)
            nc.vector.tensor_scalar_mul(negmx[:], tmb[:], -1.0)
            e = smp.tile([128, KM], f32)
            esum = smp.tile([128, 1], f32)
            nc.scalar.activation(e[:], rf[:], AF.Exp, bias=negmx[:],
                                 accum_out=esum[:])
            esb = smp.tile([128, 1], f32)
            nc.gpsimd.partition_all_reduce(esb[:], esum[:], 128,
                                           bass_isa.ReduceOp.add)
            rsb = smp.tile([128, 1], f32)
            nc.vector.reciprocal(rsb[:], esb[:])
            pt = smp.tile([128, KM], f32)
            nc.vector.tensor_scalar_mul(pt[:], e[:], rsb[:])
            pt_b = smp.tile([128, KM], bf16)
            nc.vector.tensor_copy(pt_b[:], pt[:])

            # ---- u = mem_hat @ p, FULL D on every core ----
            pu = ps_u.tile([128, KZ], f32)
            n_mm = 0
            N_MM_TOT = KZ * KM
            for dt in range(KZ):
                for kc in range(KM):
                    nc.tensor.matmul(
                        pu[:, dt:dt + 1],
                        memT[:, kc, 128 * dt:128 * (dt + 1)],
                        pt_b[:, kc:kc + 1],
                        start=(n_mm == 0), stop=(n_mm == N_MM_TOT - 1),
                    )
                    n_mm += 1
            u_b = smp.tile([128, KZ], bf16)
            nc.vector.tensor_copy(u_b[:], pu[:])

            # ---- z = s_q + H[rows_c, :] @ u, local rows only ----
            pz = ps_z.tile([128, KD], f32)
            n_mm = 0
            N_MM_TOT = KZ * KD
            for dt in range(KD):
                for kc in range(KZ):
                    nc.tensor.matmul(
                        pz[:, dt:dt + 1],
                        hT[:, kc, 128 * dt:128 * (dt + 1)],
                        u_b[:, kc:kc + 1],
                        start=(n_mm == 0), stop=(n_mm == N_MM_TOT - 1),
                    )
                    n_mm += 1
            zf = smp.tile([128, KD], f32)
            nc.vector.tensor_add(zf[:], s_q[:], pz[:])

            # ---- prelu(z) = max(z,0) + a*min(z,0), all on the DVE ----
            pos = smp.tile([128, KD], f32)
            nc.vector.tensor_scalar_max(pos[:], zf[:], 0.0)
            negs = smp.tile([128, KD], f32)
            nc.vector.tensor_scalar(negs[:], zf[:], 0.0, ab[:],
                                    mybir.AluOpType.min,
                                    mybir.AluOpType.mult)
            pzz = smp.tile([128, KD], f32)
            nc.vector.tensor_add(pzz[:], pos[:], negs[:])
            pz_b = smp.tile([128, KD], bf16)
            pzb_op = nc.vector.tensor_copy(pz_b[:], pzz[:])

            # ---- y partial = R[:, rows_c] @ prelu(z_c): full-D partial,
            # summed across cores on the host ----
            py = ps_y.tile([128, KZ], f32)
            n_mm = 0
            N_MM_TOT = KZ * KD
            for kc in range(KD):
                for dt in range(KZ):
                    nc.tensor.matmul(
                        py[:, dt:dt + 1],
                        rT[:, kc, 128 * dt:128 * (dt + 1)],
                        pz_b[:, kc:kc + 1],
                        start=(n_mm == 0), stop=(n_mm == N_MM_TOT - 1),
                    )
                    n_mm += 1

            # consume heater + warmup results (anti-DCE) via dummy output.
            # Pinned after the pz_b cast so they run on the DVE during the y
            # matmuls instead of serializing after the y copy.
            hw_sb = smp.tile([1, 8], f32)
            hw0 = nc.vector.memset(hw_sb[:], 0.0)
            hw1 = nc.vector.tensor_copy(hw_sb[:, 0:4], ph[0:1, 0:4])
            hw3 = nc.vector.tensor_copy(hw_sb[:, 5:6], pw3[0:1, 0:1])
            for hw_op in (hw0, hw1, hw3):
                tile_mod.add_dep_helper(hw_op.ins, pzb_op.ins, sync=False,
                                        reason="anti-DCE copies during y")
            nc.sync.dma_start(hw_out[:], hw_sb[:])

            y_sb = smp.tile([128, KZ], f32)
            nc.vector.tensor_copy(y_sb[:], py[:])
            nc.sync.dma_start(y_out[:], y_sb[:])

    nc.compile()
    return nc


def _get_module(n_cores=N_CORES, d=D, m=M, l=L):  # noqa: E741
    key = (n_cores, d, m, l)
    if key not in _CACHE:
        _CACHE[key] = _build_module(n_cores, d, m, l)
    return _CACHE[key]


def _bf(x):
    import ml_dtypes
    return np.ascontiguousarray(x).astype(ml_dtypes.bfloat16)


def _pack(x, group):
    """[n*128, e] -> [128, ...] per-partition-contiguous: rows grouped into
    chunks of `group` 128-row tiles laid side by side along the free dim."""
    n128, e = x.shape
    n = n128 // 128
    assert n % group == 0
    return np.ascontiguousarray(
        x.reshape(n // group, group, 128, e).transpose(0, 2, 1, 3)
    ).reshape((n // group) * 128, group * e)


def _make_in_maps(n_cores, d, m, l, F_q, query, memory_nodes, H, R, a_out):  # noqa: E741
    f32 = np.float32
    DL = d // n_cores
    KZ = d // 128
    KD = DL // 128
    KL = l // 128
    q2d = np.ascontiguousarray(query.reshape(KL, 128).T).astype(f32, copy=False)
    ss_full = (memory_nodes.astype(np.float64)**2).sum(axis=0).astype(f32)
    rdn = 1.0 / np.maximum(np.sqrt(ss_full), 1e-12)
    mem_hat = (memory_nodes * rdn[None, :]).astype(f32)
    memT_full = _pack(_bf(mem_hat.T), m // 128)
    FQ_N = min(8, KL)
    in_maps = []
    for c in range(n_cores):
        rows = slice(DL * c, DL * (c + 1))
        in_maps.append({
            "fqT": _pack(_bf(F_q[rows].T), FQ_N),
            "q2d": _bf(q2d),
            "memd": _pack(_bf(mem_hat[rows]), DL // 128),
            "memT": memT_full,
            "hT": _pack(_bf(H[rows].T), d // 128),
            "rT": _pack(_bf(R[:, rows].T), DL // 128),
            "ab": np.full((128, 1), a_out, f32),
        })
    return in_maps


class _PjrtRunner:
    """Cached jit(shard_map(bass_exec)) so repeat kernel() calls skip
    retracing/recompiling (bass_utils.run_bass_kernel_spmd rebuilds the jit
    closure every call)."""

    def __init__(self, nc, n_cores):
        import jax
        from jax.sharding import Mesh, PartitionSpec
        from jax.experimental.shard_map import shard_map
        from concourse import bass2jax
        import concourse.mybir as mybir

        bass2jax.install_neuronx_cc_hook()
        self.n_cores = n_cores
        part_name = (nc.partition_id_tensor.name
                     if nc.partition_id_tensor else None)
        in_names, out_names, out_avals = [], [], []
        for alloc in nc.m.functions[0].allocations:
            if not isinstance(alloc, mybir.MemoryLocationSet):
                continue
            name = alloc.memorylocations[0].name
            if alloc.kind == "ExternalInput":
                if name != part_name:
                    in_names.append(name)
            elif alloc.kind == "ExternalOutput":
                out_names.append(name)
                out_avals.append(jax.core.ShapedArray(
                    tuple(alloc.tensor_shape), mybir.dt.np(alloc.dtype)))
        self.in_names, self.out_names, self.out_avals = in_names, out_names, out_avals
        n_params = len(in_names)
        self.zero_outs = [np.zeros(a.shape, a.dtype) for a in out_avals]
        all_in_names = tuple(in_names + out_names)
        if part_name is not None:
            all_in_names = all_in_names + (part_name,)

        def _body(*args):
            operands = list(args)
            if part_name is not None:
                operands.append(bass2jax.partition_id_tensor())
            outs = bass2jax._bass_exec_p.bind(
                *operands,
                out_avals=tuple(out_avals),
                in_names=all_in_names,
                out_names=tuple(out_names),
                lowering_input_output_aliases=(),
                sim_require_finite=True,
                sim_require_nnan=True,
                nc=nc,
            )
            return tuple(outs)

        devices = jax.devices()[:n_cores]
        mesh = Mesh(np.asarray(devices), ("core",))
        n_out = len(out_names)
        self._fn = jax.jit(
            shard_map(
                _body, mesh=mesh,
                in_specs=(PartitionSpec("core"),) * (n_params + n_out),
                out_specs=(PartitionSpec("core"),) * n_out,
                check_rep=False,
            ),
            keep_unused=True,
        )

    def __call__(self, in_maps):
        n = self.n_cores
        concat_in = [
            np.concatenate([in_maps[c][name] for c in range(n)], axis=0)
            for name in self.in_names
        ]
        concat_zeros = [
            np.zeros((n * z.shape[0], *z.shape[1:]), z.dtype)
            for z in self.zero_outs
        ]
        out_arrs = self._fn(*concat_in, *concat_zeros)
        return [
            {name: np.asarray(out_arrs[i]).reshape(n, *self.out_avals[i].shape)[c]
             for i, name in enumerate(self.out_names)}
            for c in range(n)
        ]


_RUNNER = {}


def _get_runner():
    if "r" not in _RUNNER:
        _RUNNER["r"] = _PjrtRunner(_get_module(), N_CORES)
    return _RUNNER["r"]


def kernel(**inputs):
    f32 = np.float32
    F_q = np.asarray(inputs["F_q"], f32)
    query = np.asarray(inputs["query"], f32).reshape(-1)
    memory_nodes = np.asarray(inputs["memory_nodes"], f32)
    H = np.asarray(inputs["H"], f32)
    R = np.asarray(inputs["R"], f32)
    a_out = float(np.asarray(inputs["a_out"]).reshape(-1)[0])

    in_maps = _make_in_maps(N_CORES, D, M, L, F_q, query, memory_nodes,
                            H, R, a_out)
    results = _get_runner()(in_maps)
    y_pm = np.sum([results[c]["y"] for c in range(N_CORES)], axis=0)
    y = np.ascontiguousarray(y_pm.T).reshape(-1)
    return y.reshape(D, 1).astype(f32)


# revision 45
# speedup vs baseline: 1.0577x; 1.0083x over previous
"""EntNet forward kernel for 8 Trainium2 NeuronCores (Bass/Tile).

Math note: in the reference, the gated memory is
    mem = memory_nodes * (1 + sigmoid(...))
followed by per-column L2 normalization.  Since (1 + sigmoid(x)) > 0 is a
per-column positive scalar, it cancels exactly in the normalization, so the
gate g — and with it s_in, F_i, input, keys, U, V, W, a_mem (cand is dead in
the source already) — does not affect the output.  Live computation:

    s_q = F_q @ query[0]                         # [D]
    mn  = memory_nodes / max(||col||_2, 1e-12)   # [D, M] column-normalized
    p   = softmax(s_q^T @ mn)                    # [1, M]
    u   = mn @ p^T                               # [D]
    y   = R @ prelu(s_q + H @ u, a_out)          # [D, 1]

Sharding: D is row-sharded over 8 cores (rows_c = 512c:512c+512).  Each core
streams its shards of F_q^T, mem, mem^T (row shards), H^T (H column shard),
R^T (R row shard) — ~18 MiB/core in bf16.  Collectives: a tiny warm-up
AllReduce at t~0 (absorbs CC cold start + inter-core launch skew), a 4 KB
AllReduce of the r = mem^T s_q partials, and a 16 KB AllReduce of
z = s_q + H@u partials.

Precision: all matrices ship as plain bf16 (half the f32 bytes).  The final
relative error is ~3e-3 (measured vs the f32 reference), dominated by bf16
rounding of F_q/H/R; the softmax is effectively one-hot (top-2 logit gap
~80), so the p path is insensitive.  PSUM accumulation is f32.

Vectors are partition-major throughout: v[128i + p] <-> tile[p, i].
"""

import sys
import numpy as np

for _p in ("/root/.axon_site/_ro/trn_rl_repo", "/opt/trn_rl_repo"):
    if _p not in sys.path:
        sys.path.append(_p)

D, M, L = 4096, 1024, 8192
N_CORES = 8

_CACHE = {}


def _build_module(n_cores, d, m, l):  # noqa: E741
    import concourse.bacc as bacc
    import concourse.tile as tile
    import concourse.tile as tile_mod
    import concourse.mybir as mybir
    import concourse.bass_isa as bass_isa

    f32 = mybir.dt.float32
    bf16 = mybir.dt.bfloat16
    DL = d // n_cores      # local rows of D
    KD = DL // 128         # local d chunks
    KM = m // 128          # m chunks
    KZ = d // 128          # global d chunks
    KL = l // 128          # l chunks
    AF = mybir.ActivationFunctionType
    ADD = mybir.AluOpType.add
    rg = [list(range(n_cores))]

    # F_q^T streamed in chunks of FQ_N l-tiles (8 -> 1 MiB bf16 at full size)
    FQ_N = min(8, KL)
    assert KL % FQ_N == 0
    FQCH = KL // FQ_N

    nc = bacc.Bacc("TRN2", target_bir_lowering=False, debug=False,
                   enable_asserts=False, num_devices=n_cores)

    fqT_in = nc.dram_tensor("fqT", [FQCH * 128, FQ_N * DL], bf16,
                            kind="ExternalInput")
    q2d_in = nc.dram_tensor("q2d", [128, KL], bf16, kind="ExternalInput")
    memd_in = nc.dram_tensor("memd", [128, KD * m], bf16, kind="ExternalInput")
    # mem_hat^T replicated in full: u is computed on every core, so the
    # z/prelu/y chain is local and the second collective disappears
    memT_in = nc.dram_tensor("memT", [128, KM * d], bf16, kind="ExternalInput")
    hT_in = nc.dram_tensor("hT", [128, KZ * DL], bf16, kind="ExternalInput")
    rT_in = nc.dram_tensor("rT", [128, KD * d], bf16, kind="ExternalInput")
    ab_in = nc.dram_tensor("ab", [128, 1], f32, kind="ExternalInput")
    y_out = nc.dram_tensor("y", [128, KZ], f32, kind="ExternalOutput")
    hw_out = nc.dram_tensor("hw", [1, 8], f32, kind="ExternalOutput")

    with tile.TileContext(nc) as tc:
        with (
            tc.tile_pool(name="consts", bufs=1) as consts,
            tc.tile_pool(name="fq", bufs=6) as fqp,
            tc.tile_pool(name="mem", bufs=1) as memp,
            tc.tile_pool(name="big", bufs=1) as bigp,
            tc.tile_pool(name="sm", bufs=1) as smp,
            tc.tile_pool(name="scr", bufs=2) as scrp,
            tc.tile_pool(name="ps_sq", bufs=1, space="PSUM") as ps_sq,
            tc.tile_pool(name="ps_r", bufs=1, space="PSUM") as ps_r,
            tc.tile_pool(name="ps_u", bufs=1, space="PSUM") as ps_u,
            tc.tile_pool(name="ps_z", bufs=1, space="PSUM") as ps_z,
            tc.tile_pool(name="ps_y", bufs=1, space="PSUM") as ps_y,
            tc.tile_pool(name="ps_h", bufs=1, space="PSUM") as ps_h,
            tc.tile_pool(name="dram", bufs=1, space="DRAM") as dram,
        ):
            # ---- small constants (gpsimd queue, land in ~1us) ----
            q2d = consts.tile([128, KL], bf16)
            nc.gpsimd.dma_start(q2d[:], q2d_in[:])
            ab = consts.tile([128, 1], f32)
            nc.gpsimd.dma_start(ab[:], ab_in[:])

            BYP = mybir.AluOpType.bypass
            pw = consts.tile([128, 1], f32)
            nc.gpsimd.memset(pw[:], 1.0)

            # ---- phase 1: s_q = F_q @ query, streaming F_q^T over 3 queues ----
            psq = ps_sq.tile([128, KD], f32)
            fqT_r = fqT_in[:].rearrange("(i p) x -> p i x", p=128)
            engs = [nc.sync, nc.scalar]
            n_mm = 0
            N_MM_TOT = KL * KD
            last_fq_op = {}
            for i in range(FQCH):
                fq_t = fqp.tile([128, FQ_N, DL], bf16)
                last_fq_op[i % len(engs)] = engs[i % len(engs)].dma_start(
                    fq_t[:].rearrange("p a b -> p (a b)"), fqT_r[:, i, :])
                for j in range(FQ_N):
                    n = FQ_N * i + j
                    for dt in range(KD):
                        nc.tensor.matmul(
                            psq[:, dt:dt + 1],
                            fq_t[:, j, 128 * dt:128 * (dt + 1)],
                            q2d[:, n:n + 1],
                            start=(n_mm == 0), stop=(n_mm == N_MM_TOT - 1),
                        )
                        n_mm += 1
            # mem row-shard goes behind the last gpsimd fq trigger; H^T, R^T
            # and the replicated mem_hat^T land after the F stream on the two
            # big queues (memT split across both).
            memd = memp.tile([128, KD, m], bf16)
            memT = memp.tile([128, KM, d], bf16)
            md_op = nc.gpsimd.dma_start(
                memd[:].rearrange("p a b -> p (a b)"), memd_in[:])
            # warm the GpSimd partition-reduce ucode (first dispatch stalls
            # the engine ~20us) — after the memd trigger so the stall cannot
            # delay that descriptor
            pw2 = consts.tile([128, 1], f32)
            pw2_op = nc.gpsimd.partition_all_reduce(pw2[:], pw[:], 128,
                                                    bass_isa.ReduceOp.max)
            tile_mod.add_dep_helper(pw2_op.ins, md_op.ins, sync=False,
                                    reason="gpsimd warmups after mem DMA")
            pw3 = consts.tile([128, 1], f32)
            nc.gpsimd.partition_all_reduce(pw3[:], pw2[:], 128,
                                           bass_isa.ReduceOp.add)
            # the big post-stream transfers MUST stay behind the fq triggers
            # on their rings or they starve phase 1 (the scheduler hoists
            # dependency-free triggers to the ring head otherwise)
            hT = bigp.tile([128, KZ, DL], bf16)
            ht_op = nc.sync.dma_start(
                hT[:].rearrange("p a b -> p (a b)"), hT_in[:])
            rT = bigp.tile([128, KD, d], bf16)
            rt_op = nc.scalar.dma_start(
                rT[:].rearrange("p a b -> p (a b)"), rT_in[:])
            memT_f = memT[:].rearrange("p a b -> p (a b)")
            HALF = KM * d // 2
            mts_op = nc.sync.dma_start(memT_f[:, 0:HALF], memT_in[:, 0:HALF])
            mtc_op = nc.scalar.dma_start(memT_f[:, HALF:], memT_in[:, HALF:])
            tile_mod.add_dep_helper(ht_op.ins, last_fq_op[0].ins, sync=False,
                                    reason="hT after sync-ring fq stream")
            tile_mod.add_dep_helper(mts_op.ins, ht_op.ins, sync=False,
                                    reason="memT after hT on sync ring")
            tile_mod.add_dep_helper(rt_op.ins, last_fq_op[1].ins, sync=False,
                                    reason="rT after scalar-ring fq stream")
            tile_mod.add_dep_helper(mtc_op.ins, rt_op.ins, sync=False,
                                    reason="memT after rT on scalar ring")

            # ---- ACT table warmup (after the scalar-queue fq triggers; the
            # tables just have to be resident before the Exp at softmax time)
            warm = consts.tile([1, 1], f32)
            nc.gpsimd.memset(warm[:], 1.0)
            w2 = consts.tile([1, 1], f32)
            nc.scalar.activation(w2[:], warm[:], AF.Square)
            nc.scalar.activation(w2[:], warm[:], AF.Sqrt)
            nc.scalar.activation(w2[:], warm[:], AF.Exp)
            nc.scalar.activation(w2[:], warm[:], AF.Relu)

            s_q = smp.tile([128, KD], f32)
            nc.vector.tensor_copy(s_q[:], psq[:])
            sq_b = smp.tile([128, KD], bf16)
            nc.vector.tensor_copy(sq_b[:], s_q[:])

            # ---- r = mem^T @ s_q (local-d partial) ----
            pr = ps_r.tile([128, KM], f32)
            n_mm = 0
            N_MM_TOT = KM * KD
            for mt in range(KM):
                for kc in range(KD):
                    last_r_mm = nc.tensor.matmul(
                        pr[:, mt:mt + 1],
                        memd[:, kc, 128 * mt:128 * (mt + 1)],
                        sq_b[:, kc:kc + 1],
                        start=(n_mm == 0), stop=(n_mm == N_MM_TOT - 1),
                    )
                    n_mm += 1

            # PE heater: keep the PE warm through the r-AllGather wait
            HW_N = min(512, m)
            ph = ps_h.tile([1, HW_N], f32)
            h1 = []
            for k in range(30):
                h1.append(nc.tensor.matmul(
                    ph[:, :], memd[:, 0, k:k + 1], memd[:, 0, 0:HW_N],
                    start=(k == 0), stop=(k == 29)))
            tile_mod.add_dep_helper(h1[0].ins, last_r_mm.ins, sync=False,
                                    reason="heater after r")

            # ---- AllGather r partials, then local tree-sum (an AllGather is
            # one mesh phase instead of AllReduce's two) ----
            r_sb = smp.tile([128, KM], f32)
            nc.vector.tensor_copy(r_sb[:], pr[:])
            cr_i = dram.tile([128, KM], f32)
            cr_o = dram.tile([n_cores * 128, KM], f32)
            nc.gpsimd.dma_start(cr_i[:], r_sb[:])
            nc.gpsimd.collective_compute(
                "AllGather", BYP, replica_groups=rg,
                ins=[cr_i[:].opt()], outs=[cr_o[:].opt()])
            rg8 = smp.tile([128, n_cores, KM], f32)
            nc.gpsimd.dma_start(
                rg8[:], cr_o[:].rearrange("(g p) k -> p g k", p=128))
            rg8f = rg8[:].rearrange("p g k -> p (g k)")
            r4 = smp.tile([128, 4 * KM], f32)
            nc.vector.tensor_add(r4[:], rg8f[:, 0:4 * KM], rg8f[:, 4 * KM:])
            r2 = smp.tile([128, 2 * KM], f32)
            nc.vector.tensor_add(r2[:], r4[:, 0:2 * KM], r4[:, 2 * KM:])
            rf = smp.tile([128, KM], f32)
            nc.vector.tensor_add(rf[:], r2[:, 0:KM], r2[:, KM:])

            # ---- softmax (partition-major); the column norms are folded
            # into memd/memT on the host, so rf already holds the logits ----
            tm = smp.tile([128, 1], f32)
            nc.vector.tensor_reduce(tm[:], rf[:], mybir.AxisListType.X,
                                    mybir.AluOpType.max)
            tmb = smp.tile([128, 1], f32)
            nc.gpsimd.partition_all_reduce(tmb[:], tm[:], 128,
                                           bass_isa.ReduceOp.max)
            negmx = smp.tile([128, 1], f32)
            nc.vector.tensor_scalar_mul(negmx[:], tmb[:], -1.0)
            e = smp.tile([128, KM], f32)
            esum = smp.tile([128, 1], f32)
            nc.scalar.activation(e[:], rf[:], AF.Exp, bias=negmx[:],
                                 accum_out=esum[:])
            esb = smp.tile([128, 1], f32)
            nc.gpsimd.partition_all_reduce(esb[:], esum[:], 128,
                                           bass_isa.ReduceOp.add)
            rsb = smp.tile([128, 1], f32)
            nc.vector.reciprocal(rsb[:], esb[:])
            pt = smp.tile([128, KM], f32)
            nc.vector.tensor_scalar_mul(pt[:], e[:], rsb[:])
            pt_b = smp.tile([128, KM], bf16)
            nc.vector.tensor_copy(pt_b[:], pt[:])

            # ---- u = mem_hat @ p, FULL D on every core ----
            pu = ps_u.tile([128, KZ], f32)
            n_mm = 0
            N_MM_TOT = KZ * KM
            for dt in range(KZ):
                for kc in range(KM):
                    nc.tensor.matmul(
                        pu[:, dt:dt + 1],
                        memT[:, kc, 128 * dt:128 * (dt + 1)],
                        pt_b[:, kc:kc + 1],
                        start=(n_mm == 0), stop=(n_mm == N_MM_TOT - 1),
                    )
                    n_mm += 1
            u_b = smp.tile([128, KZ], bf16)
            nc.vector.tensor_copy(u_b[:], pu[:])

            # ---- z = s_q + H[rows_c, :] @ u, local rows only ----
            pz = ps_z.tile([128, KD], f32)
            n_mm = 0
            N_MM_TOT = KZ * KD
            for dt in range(KD):
                for kc in range(KZ):
                    nc.tensor.matmul(
                        pz[:, dt:dt + 1],
                        hT[:, kc, 128 * dt:128 * (dt + 1)],
                        u_b[:, kc:kc + 1],
                        start=(n_mm == 0), stop=(n_mm == N_MM_TOT - 1),
                    )
                    n_mm += 1
            zf = smp.tile([128, KD], f32)
            nc.vector.tensor_add(zf[:], s_q[:], pz[:])

            # ---- prelu(z) = max(z,0) + a*min(z,0), all on the DVE ----
            pos = smp.tile([128, KD], f32)
            nc.vector.tensor_scalar_max(pos[:], zf[:], 0.0)
            negs = smp.tile([128, KD], f32)
            nc.vector.tensor_scalar(negs[:], zf[:], 0.0, ab[:],
                                    mybir.AluOpType.min,
                                    mybir.AluOpType.mult)
            pzz = smp.tile([128, KD], f32)
            nc.vector.tensor_add(pzz[:], pos[:], negs[:])
            pz_b = smp.tile([128, KD], bf16)
            pzb_op = nc.vector.tensor_copy(pz_b[:], pzz[:])

            # ---- y partial = R[:, rows_c] @ prelu(z_c): full-D partial,
            # summed across cores on the host ----
            py = ps_y.tile([128, KZ], f32)
            n_mm = 0
            N_MM_TOT = KZ * KD
            for kc in range(KD):
                for dt in range(KZ):
                    nc.tensor.matmul(
                        py[:, dt:dt + 1],
                        rT[:, kc, 128 * dt:128 * (dt + 1)],
                        pz_b[:, kc:kc + 1],
                        start=(n_mm == 0), stop=(n_mm == N_MM_TOT - 1),
                    )
                    n_mm += 1

            # consume heater + warmup results (anti-DCE) via dummy output.
            # Pinned after the pz_b cast so they run on the DVE during the y
            # matmuls instead of serializing after the y copy.
            hw_sb = smp.tile([1, 8], f32)
            hw0 = nc.vector.memset(hw_sb[:], 0.0)
            hw1 = nc.vector.tensor_copy(hw_sb[:, 0:4], ph[0:1, 0:4])
            hw3 = nc.vector.tensor_copy(hw_sb[:, 5:6], pw3[0:1, 0:1])
            for hw_op in (hw0, hw1, hw3):
                tile_mod.add_dep_helper(hw_op.ins, pzb_op.ins, sync=False,
                                        reason="anti-DCE copies during y")
            nc.sync.dma_start(hw_out[:], hw_sb[:])

            y_sb = smp.tile([128, KZ], f32)
            nc.vector.tensor_copy(y_sb[:], py[:])
            nc.sync.dma_start(y_out[:], y_sb[:])

    nc.compile()
    return nc


def _get_module(n_cores=N_CORES, d=D, m=M, l=L):  # noqa: E741
    key = (n_cores, d, m, l)
    if key not in _CACHE:
        _CACHE[key] = _build_module(n_cores, d, m, l)
    return _CACHE[key]


def _bf(x):
    import ml_dtypes
    return np.ascontiguousarray(x).astype(ml_dtypes.bfloat16)


def _pack(x, group):
    """[n*128, e] -> [128, ...] per-partition-contiguous: rows grouped into
    chunks of `group` 128-row tiles laid side by side along the free dim."""
    n128, e = x.shape
    n = n128 // 128
    assert n % group == 0
    return np.ascontiguousarray(
        x.reshape(n // group, group, 128, e).transpose(0, 2, 1, 3)
    ).reshape((n // group) * 128, group * e)


def _make_in_maps(n_cores, d, m, l, F_q, query, memory_nodes, H, R, a_out):  # noqa: E741
    f32 = np.float32
    DL = d // n_cores
    KZ = d // 128
    KD = DL // 128
    KL = l // 128
    q2d = np.ascontiguousarray(query.reshape(KL, 128).T).astype(f32, copy=False)
    ss_full = (memory_nodes.astype(np.float64)**2).sum(axis=0).astype(f32)
    rdn = 1.0 / np.maximum(np.sqrt(ss_full), 1e-12)
    mem_hat = (memory_nodes * rdn[None, :]).astype(f32)
    memT_full = _pack(_bf(mem_hat.T), m // 128)
    FQ_N = min(8, KL)
    in_maps = []
    for c in range(n_cores):
        rows = slice(DL * c, DL * (c + 1))
        in_maps.append({
            "fqT": _pack(_bf(F_q[rows].T), FQ_N),
            "q2d": _bf(q2d),
            "memd": _pack(_bf(mem_hat[rows]), DL // 128),
            "memT": memT_full,
            "hT": _pack(_bf(H[rows].T), d // 128),
            "rT": _pack(_bf(R[:, rows].T), DL // 128),
            "ab": np.full((128, 1), a_out, f32),
        })
    return in_maps


class _PjrtRunner:
    """Cached jit(shard_map(bass_exec)) so repeat kernel() calls skip
    retracing/recompiling (bass_utils.run_bass_kernel_spmd rebuilds the jit
    closure every call)."""

    def __init__(self, nc, n_cores):
        import jax
        from jax.sharding import Mesh, PartitionSpec
        from jax.experimental.shard_map import shard_map
        from concourse import bass2jax
        import concourse.mybir as mybir

        bass2jax.install_neuronx_cc_hook()
        self.n_cores = n_cores
        part_name = (nc.partition_id_tensor.name
                     if nc.partition_id_tensor else None)
        in_names, out_names, out_avals = [], [], []
        for alloc in nc.m.functions[0].allocations:
            if not isinstance(alloc, mybir.MemoryLocationSet):
                continue
            name = alloc.memorylocations[0].name
            if alloc.kind == "ExternalInput":
                if name != part_name:
                    in_names.append(name)
            elif alloc.kind == "ExternalOutput":
                out_names.append(name)
                out_avals.append(jax.core.ShapedArray(
                    tuple(alloc.tensor_shape), mybir.dt.np(alloc.dtype)))
        self.in_names, self.out_names, self.out_avals = in_names, out_names, out_avals
        n_params = len(in_names)
        self.zero_outs = [np.zeros(a.shape, a.dtype) for a in out_avals]
        all_in_names = tuple(in_names + out_names)
        if part_name is not None:
            all_in_names = all_in_names + (part_name,)

        def _body(*args):
            operands = list(args)
            if part_name is not None:
                operands.append(bass2jax.partition_id_tensor())
            outs = bass2jax._bass_exec_p.bind(
                *operands,
                out_avals=tuple(out_avals),
                in_names=all_in_names,
                out_names=tuple(out_names),
                lowering_input_output_aliases=(),
                sim_require_finite=True,
                sim_require_nnan=True,
                nc=nc,
            )
            return tuple(outs)

        devices = jax.devices()[:n_cores]
        mesh = Mesh(np.asarray(devices), ("core",))
        n_out = len(out_names)
        self._fn = jax.jit(
            shard_map(
                _body, mesh=mesh,
                in_specs=(PartitionSpec("core"),) * (n_params + n_out),
                out_specs=(PartitionSpec("core"),) * n_out,
                check_rep=False,
            ),
            keep_unused=True,
        )

    def __call__(self, in_maps):
        n = self.n_cores
        concat_in = [
            np.concatenate([in_maps[c][name] for c in range(n)], axis=0)
            for name in self.in_names
        ]
        concat_zeros = [
            np.zeros((n * z.shape[0], *z.shape[1:]), z.dtype)
            for z in self.zero_outs
        ]
        out_arrs = self._fn(*concat_in, *concat_zeros)
        return [
            {name: np.asarray(out_arrs[i]).reshape(n, *self.out_avals[i].shape)[c]
             for i, name in enumerate(self.out_names)}
            for c in range(n)
        ]


_RUNNER = {}


def _get_runner():
    if "r" not in _RUNNER:
        _RUNNER["r"] = _PjrtRunner(_get_module(), N_CORES)
    return _RUNNER["r"]


def kernel(**inputs):
    f32 = np.float32
    F_q = np.asarray(inputs["F_q"], f32)
    query = np.asarray(inputs["query"], f32).reshape(-1)
    memory_nodes = np.asarray(inputs["memory_nodes"], f32)
    H = np.asarray(inputs["H"], f32)
    R = np.asarray(inputs["R"], f32)
    a_out = float(np.asarray(inputs["a_out"]).reshape(-1)[0])

    in_maps = _make_in_maps(N_CORES, D, M, L, F_q, query, memory_nodes,
                            H, R, a_out)
    results = _get_runner()(in_maps)
    y_pm = np.sum([results[c]["y"] for c in range(N_CORES)], axis=0)
    y = np.ascontiguousarray(y_pm.T).reshape(-1)
    return y.reshape(D, 1).astype(f32)


# revision 47
# speedup vs baseline: 1.0987x; 1.0388x over previous
"""EntNet forward kernel for 8 Trainium2 NeuronCores (Bass/Tile).

Math note: in the reference, the gated memory is
    mem = memory_nodes * (1 + sigmoid(...))
followed by per-column L2 normalization.  Since (1 + sigmoid(x)) > 0 is a
per-column positive scalar, it cancels exactly in the normalization, so the
gate g — and with it s_in, F_i, input, keys, U, V, W, a_mem (cand is dead in
the source already) — does not affect the output.  Live computation:

    s_q = F_q @ query[0]                         # [D]
    mn  = memory_nodes / max(||col||_2, 1e-12)   # [D, M] column-normalized
    p   = softmax(s_q^T @ mn)                    # [1, M]
    u   = mn @ p^T                               # [D]
    y   = R @ prelu(s_q + H @ u, a_out)          # [D, 1]

Sharding: D is row-sharded over 8 cores (rows_c = 512c:512c+512).  Each core
streams its shards of F_q^T, mem, mem^T (row shards), H^T (H column shard),
R^T (R row shard) — ~18 MiB/core in bf16.  Collectives: a tiny warm-up
AllReduce at t~0 (absorbs CC cold start + inter-core launch skew), a 4 KB
AllReduce of the r = mem^T s_q partials, and a 16 KB AllReduce of
z = s_q + H@u partials.

Precision: all matrices ship as plain bf16 (half the f32 bytes).  The final
relative error is ~3e-3 (measured vs the f32 reference), dominated by bf16
rounding of F_q/H/R; the softmax is effectively one-hot (top-2 logit gap
~80), so the p path is insensitive.  PSUM accumulation is f32.

Vectors are partition-major throughout: v[128i + p] <-> tile[p, i].
"""

import sys
import numpy as np

for _p in ("/root/.axon_site/_ro/trn_rl_repo", "/opt/trn_rl_repo"):
    if _p not in sys.path:
        sys.path.append(_p)

D, M, L = 4096, 1024, 8192
N_CORES = 8

_CACHE = {}


def _build_module(n_cores, d, m, l):  # noqa: E741
    import concourse.bacc as bacc
    import concourse.tile as tile
    import concourse.tile as tile_mod
    import concourse.mybir as mybir
    import concourse.bass_isa as bass_isa

    f32 = mybir.dt.float32
    bf16 = mybir.dt.bfloat16
    DL = d // n_cores      # local rows of D
    KD = DL // 128         # local d chunks
    KM = m // 128          # m chunks
    KZ = d // 128          # global d chunks
    KL = l // 128          # l chunks
    AF = mybir.ActivationFunctionType
    ADD = mybir.AluOpType.add
    rg = [list(range(n_cores))]

    # F_q^T streamed in chunks of FQ_N l-tiles (8 -> 1 MiB bf16 at full size)
    FQ_N = min(8, KL)
    assert KL % FQ_N == 0
    FQCH = KL // FQ_N

    nc = bacc.Bacc("TRN2", target_bir_lowering=False, debug=False,
                   enable_asserts=False, num_devices=n_cores)

    fqT_in = nc.dram_tensor("fqT", [FQCH * 128, FQ_N * DL], bf16,
                            kind="ExternalInput")
    q2d_in = nc.dram_tensor("q2d", [128, KL], bf16, kind="ExternalInput")
    memd_in = nc.dram_tensor("memd", [128, KD * m], bf16, kind="ExternalInput")
    # mem_hat^T replicated in full: u is computed on every core, so the
    # z/prelu/y chain is local and the second collective disappears
    memT_in = nc.dram_tensor("memT", [128, KM * d], bf16, kind="ExternalInput")
    hT_in = nc.dram_tensor("hT", [128, KZ * DL], bf16, kind="ExternalInput")
    rT_in = nc.dram_tensor("rT", [128, KD * d], bf16, kind="ExternalInput")
    ab_in = nc.dram_tensor("ab", [128, 1], f32, kind="ExternalInput")
    y_out = nc.dram_tensor("y", [128, KZ], f32, kind="ExternalOutput")
    hw_out = nc.dram_tensor("hw", [1, 8], f32, kind="ExternalOutput")

    with tile.TileContext(nc) as tc:
        with (
            tc.tile_pool(name="consts", bufs=1) as consts,
            tc.tile_pool(name="fq", bufs=6) as fqp,
            tc.tile_pool(name="mem", bufs=1) as memp,
            tc.tile_pool(name="big", bufs=1) as bigp,
            tc.tile_pool(name="sm", bufs=1) as smp,
            tc.tile_pool(name="scr", bufs=2) as scrp,
            tc.tile_pool(name="ps_sq", bufs=1, space="PSUM") as ps_sq,
            tc.tile_pool(name="ps_r", bufs=1, space="PSUM") as ps_r,
            tc.tile_pool(name="ps_u", bufs=1, space="PSUM") as ps_u,
            tc.tile_pool(name="ps_z", bufs=1, space="PSUM") as ps_z,
            tc.tile_pool(name="ps_y", bufs=1, space="PSUM") as ps_y,
            tc.tile_pool(name="ps_h", bufs=1, space="PSUM") as ps_h,
            tc.tile_pool(name="dram", bufs=1, space="DRAM") as dram,
        ):
            # ---- small constants on the fast rings (the gpsimd ring starts
            # moving data ~15us late) ----
            q2d = consts.tile([128, KL], bf16)
            q2d_op = nc.sync.dma_start(q2d[:], q2d_in[:])
            ab = consts.tile([128, 1], f32)
            ab_op = nc.scalar.dma_start(ab[:], ab_in[:])

            BYP = mybir.AluOpType.bypass
            pw = consts.tile([128, 1], f32)
            nc.gpsimd.memset(pw[:], 1.0)

            # ---- phase 1: s_q = F_q @ query, streaming F_q^T over 3 queues ----
            psq = ps_sq.tile([128, KD], f32)
            fqT_r = fqT_in[:].rearrange("(i p) x -> p i x", p=128)
            engs = [nc.sync, nc.scalar]
            n_mm = 0
            N_MM_TOT = KL * KD
            last_fq_op = {}
            for i in range(FQCH):
                fq_t = fqp.tile([128, FQ_N, DL], bf16)
                fq_op = engs[i % len(engs)].dma_start(
                    fq_t[:].rearrange("p a b -> p (a b)"), fqT_r[:, i, :])
                if i == 0:
                    tile_mod.add_dep_helper(fq_op.ins, q2d_op.ins, sync=False,
                                            reason="q2d first on sync ring")
                if i == 1:
                    tile_mod.add_dep_helper(fq_op.ins, ab_op.ins, sync=False,
                                            reason="ab first on scalar ring")
                last_fq_op[i % len(engs)] = fq_op
                for j in range(FQ_N):
                    n = FQ_N * i + j
                    for dt in range(KD):
                        nc.tensor.matmul(
                            psq[:, dt:dt + 1],
                            fq_t[:, j, 128 * dt:128 * (dt + 1)],
                            q2d[:, n:n + 1],
                            start=(n_mm == 0), stop=(n_mm == N_MM_TOT - 1),
                        )
                        n_mm += 1
            # mem row-shard goes behind the last gpsimd fq trigger; H^T, R^T
            # and the replicated mem_hat^T land after the F stream on the two
            # big queues (memT split across both).
            memd = memp.tile([128, KD, m], bf16)
            memT = memp.tile([128, KM, d], bf16)
            md_op = nc.gpsimd.dma_start(
                memd[:].rearrange("p a b -> p (a b)"), memd_in[:])
            # warm the GpSimd partition-reduce ucode (first dispatch stalls
            # the engine ~20us) — after the memd trigger so the stall cannot
            # delay that descriptor
            pw2 = consts.tile([128, 1], f32)
            pw2_op = nc.gpsimd.partition_all_reduce(pw2[:], pw[:], 128,
                                                    bass_isa.ReduceOp.max)
            tile_mod.add_dep_helper(pw2_op.ins, md_op.ins, sync=False,
                                    reason="gpsimd warmups after mem DMA")
            pw3 = consts.tile([128, 1], f32)
            nc.gpsimd.partition_all_reduce(pw3[:], pw2[:], 128,
                                           bass_isa.ReduceOp.add)
            # the big post-stream transfers MUST stay behind the fq triggers
            # on their rings or they starve phase 1 (the scheduler hoists
            # dependency-free triggers to the ring head otherwise)
            hT = bigp.tile([128, KZ, DL], bf16)
            ht_op = nc.sync.dma_start(
                hT[:].rearrange("p a b -> p (a b)"), hT_in[:])
            rT = bigp.tile([128, KD, d], bf16)
            rt_op = nc.scalar.dma_start(
                rT[:].rearrange("p a b -> p (a b)"), rT_in[:])
            memT_f = memT[:].rearrange("p a b -> p (a b)")
            HALF = KM * d // 2
            mts_op = nc.sync.dma_start(memT_f[:, 0:HALF], memT_in[:, 0:HALF])
            mtc_op = nc.scalar.dma_start(memT_f[:, HALF:], memT_in[:, HALF:])
            tile_mod.add_dep_helper(ht_op.ins, last_fq_op[0].ins, sync=False,
                                    reason="hT after sync-ring fq stream")
            tile_mod.add_dep_helper(mts_op.ins, ht_op.ins, sync=False,
                                    reason="memT after hT on sync ring")
            tile_mod.add_dep_helper(rt_op.ins, last_fq_op[1].ins, sync=False,
                                    reason="rT after scalar-ring fq stream")
            tile_mod.add_dep_helper(mtc_op.ins, rt_op.ins, sync=False,
                                    reason="memT after rT on scalar ring")

            # ---- ACT table warmup (after the scalar-queue fq triggers; the
            # tables just have to be resident before the Exp at softmax time)
            warm = consts.tile([1, 1], f32)
            nc.gpsimd.memset(warm[:], 1.0)
            w2 = consts.tile([1, 1], f32)
            nc.scalar.activation(w2[:], warm[:], AF.Square)
            nc.scalar.activation(w2[:], warm[:], AF.Sqrt)
            nc.scalar.activation(w2[:], warm[:], AF.Exp)
            nc.scalar.activation(w2[:], warm[:], AF.Relu)

            s_q = smp.tile([128, KD], f32)
            nc.vector.tensor_copy(s_q[:], psq[:])
            sq_b = smp.tile([128, KD], bf16)
            nc.vector.tensor_copy(sq_b[:], s_q[:])

            # ---- r = mem^T @ s_q (local-d partial) ----
            pr = ps_r.tile([128, KM], f32)
            n_mm = 0
            N_MM_TOT = KM * KD
            for mt in range(KM):
                for kc in range(KD):
                    last_r_mm = nc.tensor.matmul(
                        pr[:, mt:mt + 1],
                        memd[:, kc, 128 * mt:128 * (mt + 1)],
                        sq_b[:, kc:kc + 1],
                        start=(n_mm == 0), stop=(n_mm == N_MM_TOT - 1),
                    )
                    n_mm += 1

            # PE heater: keep the PE warm through the r-AllGather wait
            HW_N = min(512, m)
            ph = ps_h.tile([1, HW_N], f32)
            h1 = []
            for k in range(30):
                h1.append(nc.tensor.matmul(
                    ph[:, :], memd[:, 0, k:k + 1], memd[:, 0, 0:HW_N],
                    start=(k == 0), stop=(k == 29)))
            tile_mod.add_dep_helper(h1[0].ins, last_r_mm.ins, sync=False,
                                    reason="heater after r")

            # ---- AllGather r partials, then local tree-sum (an AllGather is
            # one mesh phase instead of AllReduce's two) ----
            r_sb = smp.tile([128, KM], f32)
            nc.vector.tensor_copy(r_sb[:], pr[:])
            cr_i = dram.tile([128, KM], f32)
            cr_o = dram.tile([n_cores * 128, KM], f32)
            nc.gpsimd.dma_start(cr_i[:], r_sb[:])
            nc.gpsimd.collective_compute(
                "AllGather", BYP, replica_groups=rg,
                ins=[cr_i[:].opt()], outs=[cr_o[:].opt()])
            rg8 = smp.tile([128, n_cores, KM], f32)
            nc.gpsimd.dma_start(
                rg8[:], cr_o[:].rearrange("(g p) k -> p g k", p=128))
            rg8f = rg8[:].rearrange("p g k -> p (g k)")
            r4 = smp.tile([128, 4 * KM], f32)
            nc.vector.tensor_add(r4[:], rg8f[:, 0:4 * KM], rg8f[:, 4 * KM:])
            r2 = smp.tile([128, 2 * KM], f32)
            nc.vector.tensor_add(r2[:], r4[:, 0:2 * KM], r4[:, 2 * KM:])
            rf = smp.tile([128, KM], f32)
            nc.vector.tensor_add(rf[:], r2[:, 0:KM], r2[:, KM:])

            # ---- softmax (partition-major); the column norms are folded
            # into memd/memT on the host, so rf already holds the logits ----
            tm = smp.tile([128, 1], f32)
            nc.vector.tensor_reduce(tm[:], rf[:], mybir.AxisListType.X,
                                    mybir.AluOpType.max)
            tmb = smp.tile([128, 1], f32)
            nc.gpsimd.partition_all_reduce(tmb[:], tm[:], 128,
                                           bass_isa.ReduceOp.max)
            negmx = smp.tile([128, 1], f32)
            nc.vector.tensor_scalar_mul(negmx[:], tmb[:], -1.0)
            e = smp.tile([128, KM], f32)
            esum = smp.tile([128, 1], f32)
            nc.scalar.activation(e[:], rf[:], AF.Exp, bias=negmx[:],
                                 accum_out=esum[:])
            esb = smp.tile([128, 1], f32)
            nc.gpsimd.partition_all_reduce(esb[:], esum[:], 128,
                                           bass_isa.ReduceOp.add)
            rsb = smp.tile([128, 1], f32)
            nc.vector.reciprocal(rsb[:], esb[:])
            pt = smp.tile([128, KM], f32)
            nc.vector.tensor_scalar_mul(pt[:], e[:], rsb[:])
            pt_b = smp.tile([128, KM], bf16)
            nc.vector.tensor_copy(pt_b[:], pt[:])

            # ---- u = mem_hat @ p, FULL D on every core ----
            pu = ps_u.tile([128, KZ], f32)
            n_mm = 0
            N_MM_TOT = KZ * KM
            for dt in range(KZ):
                for kc in range(KM):
                    nc.tensor.matmul(
                        pu[:, dt:dt + 1],
                        memT[:, kc, 128 * dt:128 * (dt + 1)],
                        pt_b[:, kc:kc + 1],
                        start=(n_mm == 0), stop=(n_mm == N_MM_TOT - 1),
                    )
                    n_mm += 1
            u_b = smp.tile([128, KZ], bf16)
            nc.vector.tensor_copy(u_b[:], pu[:])

            # ---- z = s_q + H[rows_c, :] @ u, local rows only ----
            pz = ps_z.tile([128, KD], f32)
            n_mm = 0
            N_MM_TOT = KZ * KD
            for dt in range(KD):
                for kc in range(KZ):
                    nc.tensor.matmul(
                        pz[:, dt:dt + 1],
                        hT[:, kc, 128 * dt:128 * (dt + 1)],
                        u_b[:, kc:kc + 1],
                        start=(n_mm == 0), stop=(n_mm == N_MM_TOT - 1),
                    )
                    n_mm += 1
            zf = smp.tile([128, KD], f32)
            nc.vector.tensor_add(zf[:], s_q[:], pz[:])

            # ---- prelu(z) = max(z,0) + a*min(z,0), all on the DVE ----
            pos = smp.tile([128, KD], f32)
            nc.vector.tensor_scalar_max(pos[:], zf[:], 0.0)
            negs = smp.tile([128, KD], f32)
            nc.vector.tensor_scalar(negs[:], zf[:], 0.0, ab[:],
                                    mybir.AluOpType.min,
                                    mybir.AluOpType.mult)
            pzz = smp.tile([128, KD], f32)
            nc.vector.tensor_add(pzz[:], pos[:], negs[:])
            pz_b = smp.tile([128, KD], bf16)
            pzb_op = nc.vector.tensor_copy(pz_b[:], pzz[:])

            # ---- y partial = R[:, rows_c] @ prelu(z_c): full-D partial,
            # summed across cores on the host ----
            py = ps_y.tile([128, KZ], f32)
            n_mm = 0
            N_MM_TOT = KZ * KD
            for kc in range(KD):
                for dt in range(KZ):
                    nc.tensor.matmul(
                        py[:, dt:dt + 1],
                        rT[:, kc, 128 * dt:128 * (dt + 1)],
                        pz_b[:, kc:kc + 1],
                        start=(n_mm == 0), stop=(n_mm == N_MM_TOT - 1),
                    )
                    n_mm += 1

            # consume heater + warmup results (anti-DCE) via dummy output.
            # Pinned after the pz_b cast so they run on the DVE during the y
            # matmuls instead of serializing after the y copy.
            hw_sb = smp.tile([1, 8], f32)
            hw0 = nc.vector.memset(hw_sb[:], 0.0)
            hw1 = nc.vector.tensor_copy(hw_sb[:, 0:4], ph[0:1, 0:4])
            hw3 = nc.vector.tensor_copy(hw_sb[:, 5:6], pw3[0:1, 0:1])
            for hw_op in (hw0, hw1, hw3):
                tile_mod.add_dep_helper(hw_op.ins, pzb_op.ins, sync=False,
                                        reason="anti-DCE copies during y")
            nc.sync.dma_start(hw_out[:], hw_sb[:])

            y_sb = smp.tile([128, KZ], f32)
            nc.vector.tensor_copy(y_sb[:], py[:])
            nc.sync.dma_start(y_out[:], y_sb[:])

    nc.compile()
    return nc


def _get_module(n_cores=N_CORES, d=D, m=M, l=L):  # noqa: E741
    key = (n_cores, d, m, l)
    if key not in _CACHE:
        _CACHE[key] = _build_module(n_cores, d, m, l)
    return _CACHE[key]


def _bf(x):
    import ml_dtypes
    return np.ascontiguousarray(x).astype(ml_dtypes.bfloat16)


def _pack(x, group):
    """[n*128, e] -> [128, ...] per-partition-contiguous: rows grouped into
    chunks of `group` 128-row tiles laid side by side along the free dim."""
    n128, e = x.shape
    n = n128 // 128
    assert n % group == 0
    return np.ascontiguousarray(
        x.reshape(n // group, group, 128, e).transpose(0, 2, 1, 3)
    ).reshape((n // group) * 128, group * e)


def _make_in_maps(n_cores, d, m, l, F_q, query, memory_nodes, H, R, a_out):  # noqa: E741
    f32 = np.float32
    DL = d // n_cores
    KZ = d // 128
    KD = DL // 128
    KL = l // 128
    q2d = np.ascontiguousarray(query.reshape(KL, 128).T).astype(f32, copy=False)
    ss_full = (memory_nodes.astype(np.float64)**2).sum(axis=0).astype(f32)
    rdn = 1.0 / np.maximum(np.sqrt(ss_full), 1e-12)
    mem_hat = (memory_nodes * rdn[None, :]).astype(f32)
    memT_full = _pack(_bf(mem_hat.T), m // 128)
    FQ_N = min(8, KL)
    in_maps = []
    for c in range(n_cores):
        rows = slice(DL * c, DL * (c + 1))
        in_maps.append({
            "fqT": _pack(_bf(F_q[rows].T), FQ_N),
            "q2d": _bf(q2d),
            "memd": _pack(_bf(mem_hat[rows]), DL // 128),
            "memT": memT_full,
            "hT": _pack(_bf(H[rows].T), d // 128),
            "rT": _pack(_bf(R[:, rows].T), DL // 128),
            "ab": np.full((128, 1), a_out, f32),
        })
    return in_maps


class _PjrtRunner:
    """Cached jit(shard_map(bass_exec)) so repeat kernel() calls skip
    retracing/recompiling (bass_utils.run_bass_kernel_spmd rebuilds the jit
    closure every call)."""

    def __init__(self, nc, n_cores):
        import jax
        from jax.sharding import Mesh, PartitionSpec
        from jax.experimental.shard_map import shard_map
        from concourse import bass2jax
        import concourse.mybir as mybir

        bass2jax.install_neuronx_cc_hook()
        self.n_cores = n_cores
        part_name = (nc.partition_id_tensor.name
                     if nc.partition_id_tensor else None)
        in_names, out_names, out_avals = [], [], []
        for alloc in nc.m.functions[0].allocations:
            if not isinstance(alloc, mybir.MemoryLocationSet):
                continue
            name = alloc.memorylocations[0].name
            if alloc.kind == "ExternalInput":
                if name != part_name:
                    in_names.append(name)
            elif alloc.kind == "ExternalOutput":
                out_names.append(name)
                out_avals.append(jax.core.ShapedArray(
                    tuple(alloc.tensor_shape), mybir.dt.np(alloc.dtype)))
        self.in_names, self.out_names, self.out_avals = in_names, out_names, out_avals
        n_params = len(in_names)
        self.zero_outs = [np.zeros(a.shape, a.dtype) for a in out_avals]
        all_in_names = tuple(in_names + out_names)
        if part_name is not None:
            all_in_names = all_in_names + (part_name,)

        def _body(*args):
            operands = list(args)
            if part_name is not None:
                operands.append(bass2jax.partition_id_tensor())
            outs = bass2jax._bass_exec_p.bind(
                *operands,
                out_avals=tuple(out_avals),
                in_names=all_in_names,
                out_names=tuple(out_names),
                lowering_input_output_aliases=(),
                sim_require_finite=True,
                sim_require_nnan=True,
                nc=nc,
            )
            return tuple(outs)

        devices = jax.devices()[:n_cores]
        mesh = Mesh(np.asarray(devices), ("core",))
        n_out = len(out_names)
        self._fn = jax.jit(
            shard_map(
                _body, mesh=mesh,
                in_specs=(PartitionSpec("core"),) * (n_params + n_out),
                out_specs=(PartitionSpec("core"),) * n_out,
                check_rep=False,
            ),
            keep_unused=True,
        )

    def __call__(self, in_maps):
        n = self.n_cores
        concat_in = [
            np.concatenate([in_maps[c][name] for c in range(n)], axis=0)
            for name in self.in_names
        ]
        concat_zeros = [
            np.zeros((n * z.shape[0], *z.shape[1:]), z.dtype)
            for z in self.zero_outs
        ]
        out_arrs = self._fn(*concat_in, *concat_zeros)
        return [
            {name: np.asarray(out_arrs[i]).reshape(n, *self.out_avals[i].shape)[c]
             for i, name in enumerate(self.out_names)}
            for c in range(n)
        ]


_RUNNER = {}


def _get_runner():
    if "r" not in _RUNNER:
        _RUNNER["r"] = _PjrtRunner(_get_module(), N_CORES)
    return _RUNNER["r"]


def kernel(**inputs):
    f32 = np.float32
    F_q = np.asarray(inputs["F_q"], f32)
    query = np.asarray(inputs["query"], f32).reshape(-1)
    memory_nodes = np.asarray(inputs["memory_nodes"], f32)
    H = np.asarray(inputs["H"], f32)
    R = np.asarray(inputs["R"], f32)
    a_out = float(np.asarray(inputs["a_out"]).reshape(-1)[0])

    in_maps = _make_in_maps(N_CORES, D, M, L, F_q, query, memory_nodes,
                            H, R, a_out)
    results = _get_runner()(in_maps)
    y_pm = np.sum([results[c]["y"] for c in range(N_CORES)], axis=0)
    y = np.ascontiguousarray(y_pm.T).reshape(-1)
    return y.reshape(D, 1).astype(f32)


# revision 49
# speedup vs baseline: 1.2101x; 1.1013x over previous
"""EntNet forward kernel for 8 Trainium2 NeuronCores (Bass/Tile).

Math note: in the reference, the gated memory is
    mem = memory_nodes * (1 + sigmoid(...))
followed by per-column L2 normalization.  Since (1 + sigmoid(x)) > 0 is a
per-column positive scalar, it cancels exactly in the normalization, so the
gate g — and with it s_in, F_i, input, keys, U, V, W, a_mem (cand is dead in
the source already) — does not affect the output.  Live computation:

    s_q = F_q @ query[0]                         # [D]
    mn  = memory_nodes / max(||col||_2, 1e-12)   # [D, M] column-normalized
    p   = softmax(s_q^T @ mn)                    # [1, M]
    u   = mn @ p^T                               # [D]
    y   = R @ prelu(s_q + H @ u, a_out)          # [D, 1]

Sharding: D is row-sharded over 8 cores (rows_c = 512c:512c+512).  Each core
streams its shards of F_q^T, mem, mem^T (row shards), H^T (H column shard),
R^T (R row shard) — ~18 MiB/core in bf16.  Collectives: a tiny warm-up
AllReduce at t~0 (absorbs CC cold start + inter-core launch skew), a 4 KB
AllReduce of the r = mem^T s_q partials, and a 16 KB AllReduce of
z = s_q + H@u partials.

Precision: all matrices ship as plain bf16 (half the f32 bytes).  The final
relative error is ~3e-3 (measured vs the f32 reference), dominated by bf16
rounding of F_q/H/R; the softmax is effectively one-hot (top-2 logit gap
~80), so the p path is insensitive.  PSUM accumulation is f32.

Vectors are partition-major throughout: v[128i + p] <-> tile[p, i].
"""

import sys
import numpy as np

for _p in ("/root/.axon_site/_ro/trn_rl_repo", "/opt/trn_rl_repo"):
    if _p not in sys.path:
        sys.path.append(_p)

D, M, L = 4096, 1024, 8192
N_CORES = 8

_CACHE = {}


def _build_module(n_cores, d, m, l):  # noqa: E741
    import concourse.bacc as bacc
    import concourse.tile as tile
    import concourse.tile as tile_mod
    import concourse.mybir as mybir
    import concourse.bass_isa as bass_isa

    f32 = mybir.dt.float32
    bf16 = mybir.dt.bfloat16
    DL = d // n_cores      # local rows of D
    KD = DL // 128         # local d chunks
    KM = m // 128          # m chunks
    KZ = d // 128          # global d chunks
    KL = l // 128          # l chunks
    AF = mybir.ActivationFunctionType
    ADD = mybir.AluOpType.add
    rg = [list(range(n_cores))]

    # F_q^T streamed in chunks of FQ_N l-tiles (8 -> 1 MiB bf16 at full size)
    FQ_N = min(8, KL)
    assert KL % FQ_N == 0
    FQCH = KL // FQ_N

    nc = bacc.Bacc("TRN2", target_bir_lowering=False, debug=False,
                   enable_asserts=False, num_devices=n_cores)

    fqT_in = nc.dram_tensor("fqT", [FQCH * 128, FQ_N * DL], bf16,
                            kind="ExternalInput")
    q2d_in = nc.dram_tensor("q2d", [128, KL], bf16, kind="ExternalInput")
    memd_in = nc.dram_tensor("memd", [128, KD * m], bf16, kind="ExternalInput")
    # mem_hat^T replicated in full: u is computed on every core, so the
    # z/prelu/y chain is local and the second collective disappears
    memT_in = nc.dram_tensor("memT", [128, KM * d], bf16, kind="ExternalInput")
    hT_in = nc.dram_tensor("hT", [128, KZ * DL], bf16, kind="ExternalInput")
    rT_in = nc.dram_tensor("rT", [128, KD * d], bf16, kind="ExternalInput")
    ab_in = nc.dram_tensor("ab", [128, 1], f32, kind="ExternalInput")
    y_out = nc.dram_tensor("y", [128, KZ], f32, kind="ExternalOutput")
    hw_out = nc.dram_tensor("hw", [1, 8], f32, kind="ExternalOutput")

    with tile.TileContext(nc) as tc:
        with (
            tc.tile_pool(name="consts", bufs=1) as consts,
            tc.tile_pool(name="fq", bufs=6) as fqp,
            tc.tile_pool(name="mem", bufs=1) as memp,
            tc.tile_pool(name="big", bufs=1) as bigp,
            tc.tile_pool(name="sm", bufs=1) as smp,
            tc.tile_pool(name="scr", bufs=2) as scrp,
            tc.tile_pool(name="ps_sq", bufs=1, space="PSUM") as ps_sq,
            tc.tile_pool(name="ps_r", bufs=1, space="PSUM") as ps_r,
            tc.tile_pool(name="ps_u", bufs=1, space="PSUM") as ps_u,
            tc.tile_pool(name="ps_z", bufs=1, space="PSUM") as ps_z,
            tc.tile_pool(name="ps_y", bufs=1, space="PSUM") as ps_y,
            tc.tile_pool(name="ps_h", bufs=1, space="PSUM") as ps_h,
            tc.tile_pool(name="dram", bufs=1, space="DRAM") as dram,
        ):
            # ---- small constants on the fast rings (the gpsimd ring starts
            # moving data ~15us late) ----
            q2d = consts.tile([128, KL], bf16)
            q2d_op = nc.sync.dma_start(q2d[:], q2d_in[:])
            ab = consts.tile([128, 1], f32)
            ab_op = nc.scalar.dma_start(ab[:], ab_in[:])

            BYP = mybir.AluOpType.bypass
            pw = consts.tile([128, 1], f32)
            nc.gpsimd.memset(pw[:], 1.0)

            # ---- phase 1: s_q = F_q @ query, streaming F_q^T over 3 queues ----
            psq = ps_sq.tile([128, KD], f32)
            fqT_r = fqT_in[:].rearrange("(i p) x -> p i x", p=128)
            engs = [nc.sync, nc.scalar]
            n_mm = 0
            N_MM_TOT = KL * KD
            last_fq_op = {}
            for i in range(FQCH):
                fq_t = fqp.tile([128, FQ_N, DL], bf16)
                fq_op = engs[i % len(engs)].dma_start(
                    fq_t[:].rearrange("p a b -> p (a b)"), fqT_r[:, i, :])
                if i == 0:
                    tile_mod.add_dep_helper(fq_op.ins, q2d_op.ins, sync=False,
                                            reason="q2d first on sync ring")
                if i == 1:
                    tile_mod.add_dep_helper(fq_op.ins, ab_op.ins, sync=False,
                                            reason="ab first on scalar ring")
                last_fq_op[i % len(engs)] = fq_op
                for j in range(FQ_N):
                    n = FQ_N * i + j
                    for dt in range(KD):
                        last_fq_mm = nc.tensor.matmul(
                            psq[:, dt:dt + 1],
                            fq_t[:, j, 128 * dt:128 * (dt + 1)],
                            q2d[:, n:n + 1],
                            start=(n_mm == 0), stop=(n_mm == N_MM_TOT - 1),
                        )
                        n_mm += 1
            # mem row-shard goes behind the last gpsimd fq trigger; H^T, R^T
            # and the replicated mem_hat^T land after the F stream on the two
            # big queues (memT split across both).
            memd = memp.tile([128, KD, m], bf16)
            memT = memp.tile([128, KM, d], bf16)
            md_op = nc.gpsimd.dma_start(
                memd[:].rearrange("p a b -> p (a b)"), memd_in[:])
            # warm the GpSimd partition-reduce ucode (first dispatch stalls
            # the engine ~20us) — after the memd trigger so the stall cannot
            # delay that descriptor
            pw2 = consts.tile([128, 1], f32)
            pw2_op = nc.gpsimd.partition_all_reduce(pw2[:], pw[:], 128,
                                                    bass_isa.ReduceOp.max)
            tile_mod.add_dep_helper(pw2_op.ins, md_op.ins, sync=False,
                                    reason="gpsimd warmups after mem DMA")
            pw3 = consts.tile([128, 1], f32)
            nc.gpsimd.partition_all_reduce(pw3[:], pw2[:], 128,
                                           bass_isa.ReduceOp.add)
            # the big post-stream transfers share one ~350GB/s HBM port with
            # the F stream; gate them on the END of phase 1 (true semaphore
            # dep on the last F matmul) so they cannot starve it
            hT = bigp.tile([128, KZ, DL], bf16)
            ht_op = nc.sync.dma_start(
                hT[:].rearrange("p a b -> p (a b)"), hT_in[:])
            rT = bigp.tile([128, KD, d], bf16)
            rt_op = nc.scalar.dma_start(
                rT[:].rearrange("p a b -> p (a b)"), rT_in[:])
            memT_f = memT[:].rearrange("p a b -> p (a b)")
            HALF = KM * d // 2
            mts_op = nc.sync.dma_start(memT_f[:, 0:HALF], memT_in[:, 0:HALF])
            mtc_op = nc.scalar.dma_start(memT_f[:, HALF:], memT_in[:, HALF:])
            for _op in (ht_op, rt_op):
                tile_mod.add_dep_helper(_op.ins, last_fq_mm.ins, sync=True,
                                        reason="bulk DMA after the F stream")
            tile_mod.add_dep_helper(mts_op.ins, ht_op.ins, sync=False,
                                    reason="memT after hT on sync ring")
            tile_mod.add_dep_helper(mtc_op.ins, rt_op.ins, sync=False,
                                    reason="memT after rT on scalar ring")

            # ---- ACT table warmup (after the scalar-queue fq triggers; the
            # tables just have to be resident before the Exp at softmax time)
            warm = consts.tile([1, 1], f32)
            nc.gpsimd.memset(warm[:], 1.0)
            w2 = consts.tile([1, 1], f32)
            nc.scalar.activation(w2[:], warm[:], AF.Square)
            nc.scalar.activation(w2[:], warm[:], AF.Sqrt)
            nc.scalar.activation(w2[:], warm[:], AF.Exp)
            nc.scalar.activation(w2[:], warm[:], AF.Relu)

            s_q = smp.tile([128, KD], f32)
            nc.vector.tensor_copy(s_q[:], psq[:])
            sq_b = smp.tile([128, KD], bf16)
            nc.vector.tensor_copy(sq_b[:], s_q[:])

            # ---- r = mem^T @ s_q (local-d partial) ----
            pr = ps_r.tile([128, KM], f32)
            n_mm = 0
            N_MM_TOT = KM * KD
            for mt in range(KM):
                for kc in range(KD):
                    last_r_mm = nc.tensor.matmul(
                        pr[:, mt:mt + 1],
                        memd[:, kc, 128 * mt:128 * (mt + 1)],
                        sq_b[:, kc:kc + 1],
                        start=(n_mm == 0), stop=(n_mm == N_MM_TOT - 1),
                    )
                    n_mm += 1

            # PE heater: keep the PE warm through the r-AllGather wait
            HW_N = min(512, m)
            ph = ps_h.tile([1, HW_N], f32)
            h1 = []
            for k in range(30):
                h1.append(nc.tensor.matmul(
                    ph[:, :], memd[:, 0, k:k + 1], memd[:, 0, 0:HW_N],
                    start=(k == 0), stop=(k == 29)))
            tile_mod.add_dep_helper(h1[0].ins, last_r_mm.ins, sync=False,
                                    reason="heater after r")

            # ---- AllGather r partials, then local tree-sum (an AllGather is
            # one mesh phase instead of AllReduce's two) ----
            r_sb = smp.tile([128, KM], f32)
            nc.vector.tensor_copy(r_sb[:], pr[:])
            cr_i = dram.tile([128, KM], f32)
            cr_o = dram.tile([n_cores * 128, KM], f32)
            nc.gpsimd.dma_start(cr_i[:], r_sb[:])
            nc.gpsimd.collective_compute(
                "AllGather", BYP, replica_groups=rg,
                ins=[cr_i[:].opt()], outs=[cr_o[:].opt()])
            rg8 = smp.tile([128, n_cores, KM], f32)
            nc.gpsimd.dma_start(
                rg8[:], cr_o[:].rearrange("(g p) k -> p g k", p=128))
            rg8f = rg8[:].rearrange("p g k -> p (g k)")
            r4 = smp.tile([128, 4 * KM], f32)
            nc.vector.tensor_add(r4[:], rg8f[:, 0:4 * KM], rg8f[:, 4 * KM:])
            r2 = smp.tile([128, 2 * KM], f32)
            nc.vector.tensor_add(r2[:], r4[:, 0:2 * KM], r4[:, 2 * KM:])
            rf = smp.tile([128, KM], f32)
            nc.vector.tensor_add(rf[:], r2[:, 0:KM], r2[:, KM:])

            # ---- softmax (partition-major); the column norms are folded
            # into memd/memT on the host, so rf already holds the logits ----
            tm = smp.tile([128, 1], f32)
            nc.vector.tensor_reduce(tm[:], rf[:], mybir.AxisListType.X,
                                    mybir.AluOpType.max)
            tmb = smp.tile([128, 1], f32)
            nc.gpsimd.partition_all_reduce(tmb[:], tm[:], 128,
                                           bass_isa.ReduceOp.max)
            negmx = smp.tile([128, 1], f32)
            nc.vector.tensor_scalar_mul(negmx[:], tmb[:], -1.0)
            e = smp.tile([128, KM], f32)
            esum = smp.tile([128, 1], f32)
            nc.scalar.activation(e[:], rf[:], AF.Exp, bias=negmx[:],
                                 accum_out=esum[:])
            esb = smp.tile([128, 1], f32)
            nc.gpsimd.partition_all_reduce(esb[:], esum[:], 128,
                                           bass_isa.ReduceOp.add)
            rsb = smp.tile([128, 1], f32)
            nc.vector.reciprocal(rsb[:], esb[:])
            pt = smp.tile([128, KM], f32)
            nc.vector.tensor_scalar_mul(pt[:], e[:], rsb[:])
            pt_b = smp.tile([128, KM], bf16)
            nc.vector.tensor_copy(pt_b[:], pt[:])

            # ---- u = mem_hat @ p, FULL D on every core ----
            pu = ps_u.tile([128, KZ], f32)
            n_mm = 0
            N_MM_TOT = KZ * KM
            for dt in range(KZ):
                for kc in range(KM):
                    nc.tensor.matmul(
                        pu[:, dt:dt + 1],
                        memT[:, kc, 128 * dt:128 * (dt + 1)],
                        pt_b[:, kc:kc + 1],
                        start=(n_mm == 0), stop=(n_mm == N_MM_TOT - 1),
                    )
                    n_mm += 1
            u_b = smp.tile([128, KZ], bf16)
            nc.vector.tensor_copy(u_b[:], pu[:])

            # ---- z = s_q + H[rows_c, :] @ u, local rows only ----
            pz = ps_z.tile([128, KD], f32)
            n_mm = 0
            N_MM_TOT = KZ * KD
            for dt in range(KD):
                for kc in range(KZ):
                    nc.tensor.matmul(
                        pz[:, dt:dt + 1],
                        hT[:, kc, 128 * dt:128 * (dt + 1)],
                        u_b[:, kc:kc + 1],
                        start=(n_mm == 0), stop=(n_mm == N_MM_TOT - 1),
                    )
                    n_mm += 1
            zf = smp.tile([128, KD], f32)
            nc.vector.tensor_add(zf[:], s_q[:], pz[:])

            # ---- prelu(z) = max(z,0) + a*min(z,0), all on the DVE ----
            pos = smp.tile([128, KD], f32)
            nc.vector.tensor_scalar_max(pos[:], zf[:], 0.0)
            negs = smp.tile([128, KD], f32)
            nc.vector.tensor_scalar(negs[:], zf[:], 0.0, ab[:],
                                    mybir.AluOpType.min,
                                    mybir.AluOpType.mult)
            pzz = smp.tile([128, KD], f32)
            nc.vector.tensor_add(pzz[:], pos[:], negs[:])
            pz_b = smp.tile([128, KD], bf16)
            pzb_op = nc.vector.tensor_copy(pz_b[:], pzz[:])

            # ---- y partial = R[:, rows_c] @ prelu(z_c): full-D partial,
            # summed across cores on the host ----
            py = ps_y.tile([128, KZ], f32)
            n_mm = 0
            N_MM_TOT = KZ * KD
            for kc in range(KD):
                for dt in range(KZ):
                    nc.tensor.matmul(
                        py[:, dt:dt + 1],
                        rT[:, kc, 128 * dt:128 * (dt + 1)],
                        pz_b[:, kc:kc + 1],
                        start=(n_mm == 0), stop=(n_mm == N_MM_TOT - 1),
                    )
                    n_mm += 1

            # consume heater + warmup results (anti-DCE) via dummy output.
            # Pinned after the pz_b cast so they run on the DVE during the y
            # matmuls instead of serializing after the y copy.
            hw_sb = smp.tile([1, 8], f32)
            hw0 = nc.vector.memset(hw_sb[:], 0.0)
            hw1 = nc.vector.tensor_copy(hw_sb[:, 0:4], ph[0:1, 0:4])
            hw3 = nc.vector.tensor_copy(hw_sb[:, 5:6], pw3[0:1, 0:1])
            for hw_op in (hw0, hw1, hw3):
                tile_mod.add_dep_helper(hw_op.ins, pzb_op.ins, sync=False,
                                        reason="anti-DCE copies during y")
            nc.sync.dma_start(hw_out[:], hw_sb[:])

            y_sb = smp.tile([128, KZ], f32)
            nc.vector.tensor_copy(y_sb[:], py[:])
            nc.sync.dma_start(y_out[:], y_sb[:])

    nc.compile()
    return nc


def _get_module(n_cores=N_CORES, d=D, m=M, l=L):  # noqa: E741
    key = (n_cores, d, m, l)
    if key not in _CACHE:
        _CACHE[key] = _build_module(n_cores, d, m, l)
    return _CACHE[key]


def _bf(x):
    import ml_dtypes
    return np.ascontiguousarray(x).astype(ml_dtypes.bfloat16)


def _pack(x, group):
    """[n*128, e] -> [128, ...] per-partition-contiguous: rows grouped into
    chunks of `group` 128-row tiles laid side by side along the free dim."""
    n128, e = x.shape
    n = n128 // 128
    assert n % group == 0
    return np.ascontiguousarray(
        x.reshape(n // group, group, 128, e).transpose(0, 2, 1, 3)
    ).reshape((n // group) * 128, group * e)


def _make_in_maps(n_cores, d, m, l, F_q, query, memory_nodes, H, R, a_out):  # noqa: E741
    f32 = np.float32
    DL = d // n_cores
    KZ = d // 128
    KD = DL // 128
    KL = l // 128
    q2d = np.ascontiguousarray(query.reshape(KL, 128).T).astype(f32, copy=False)
    ss_full = (memory_nodes.astype(np.float64)**2).sum(axis=0).astype(f32)
    rdn = 1.0 / np.maximum(np.sqrt(ss_full), 1e-12)
    mem_hat = (memory_nodes * rdn[None, :]).astype(f32)
    memT_full = _pack(_bf(mem_hat.T), m // 128)
    FQ_N = min(8, KL)
    in_maps = []
    for c in range(n_cores):
        rows = slice(DL * c, DL * (c + 1))
        in_maps.append({
            "fqT": _pack(_bf(F_q[rows].T), FQ_N),
            "q2d": _bf(q2d),
            "memd": _pack(_bf(mem_hat[rows]), DL // 128),
            "memT": memT_full,
            "hT": _pack(_bf(H[rows].T), d // 128),
            "rT": _pack(_bf(R[:, rows].T), DL // 128),
            "ab": np.full((128, 1), a_out, f32),
        })
    return in_maps


class _PjrtRunner:
    """Cached jit(shard_map(bass_exec)) so repeat kernel() calls skip
    retracing/recompiling (bass_utils.run_bass_kernel_spmd rebuilds the jit
    closure every call)."""

    def __init__(self, nc, n_cores):
        import jax
        from jax.sharding import Mesh, PartitionSpec
        from jax.experimental.shard_map import shard_map
        from concourse import bass2jax
        import concourse.mybir as mybir

        bass2jax.install_neuronx_cc_hook()
        self.n_cores = n_cores
        part_name = (nc.partition_id_tensor.name
                     if nc.partition_id_tensor else None)
        in_names, out_names, out_avals = [], [], []
        for alloc in nc.m.functions[0].allocations:
            if not isinstance(alloc, mybir.MemoryLocationSet):
                continue
            name = alloc.memorylocations[0].name
            if alloc.kind == "ExternalInput":
                if name != part_name:
                    in_names.append(name)
            elif alloc.kind == "ExternalOutput":
                out_names.append(name)
                out_avals.append(jax.core.ShapedArray(
                    tuple(alloc.tensor_shape), mybir.dt.np(alloc.dtype)))
        self.in_names, self.out_names, self.out_avals = in_names, out_names, out_avals
        n_params = len(in_names)
        self.zero_outs = [np.zeros(a.shape, a.dtype) for a in out_avals]
        all_in_names = tuple(in_names + out_names)
        if part_name is not None:
            all_in_names = all_in_names + (part_name,)

        def _body(*args):
            operands = list(args)
            if part_name is not None:
                operands.append(bass2jax.partition_id_tensor())
            outs = bass2jax._bass_exec_p.bind(
                *operands,
                out_avals=tuple(out_avals),
                in_names=all_in_names,
                out_names=tuple(out_names),
                lowering_input_output_aliases=(),
                sim_require_finite=True,
                sim_require_nnan=True,
                nc=nc,
            )
            return tuple(outs)

        devices = jax.devices()[:n_cores]
        mesh = Mesh(np.asarray(devices), ("core",))
        n_out = len(out_names)
        self._fn = jax.jit(
            shard_map(
                _body, mesh=mesh,
                in_specs=(PartitionSpec("core"),) * (n_params + n_out),
                out_specs=(PartitionSpec("core"),) * n_out,
                check_rep=False,
            ),
            keep_unused=True,
        )

    def __call__(self, in_maps):
        n = self.n_cores
        concat_in = [
            np.concatenate([in_maps[c][name] for c in range(n)], axis=0)
            for name in self.in_names
        ]
        concat_zeros = [
            np.zeros((n * z.shape[0], *z.shape[1:]), z.dtype)
            for z in self.zero_outs
        ]
        out_arrs = self._fn(*concat_in, *concat_zeros)
        return [
            {name: np.asarray(out_arrs[i]).reshape(n, *self.out_avals[i].shape)[c]
             for i, name in enumerate(self.out_names)}
            for c in range(n)
        ]


_RUNNER = {}


def _get_runner():
    if "r" not in _RUNNER:
        _RUNNER["r"] = _PjrtRunner(_get_module(), N_CORES)
    return _RUNNER["r"]


def kernel(**inputs):
    f32 = np.float32
    F_q = np.asarray(inputs["F_q"], f32)
    query = np.asarray(inputs["query"], f32).reshape(-1)
    memory_nodes = np.asarray(inputs["memory_nodes"], f32)
    H = np.asarray(inputs["H"], f32)
    R = np.asarray(inputs["R"], f32)
    a_out = float(np.asarray(inputs["a_out"]).reshape(-1)[0])

    in_maps = _make_in_maps(N_CORES, D, M, L, F_q, query, memory_nodes,
                            H, R, a_out)
    results = _get_runner()(in_maps)
    y_pm = np.sum([results[c]["y"] for c in range(N_CORES)], axis=0)
    y = np.ascontiguousarray(y_pm.T).reshape(-1)
    return y.reshape(D, 1).astype(f32)
